# revision 1
# baseline (speedup 1.0000x reference)
"""Trainium2 Bass kernel for nn_DecoderBlock (B=1, S=4096, D=1024, H=16, dh=64).

Strategy (8 NeuronCores, SPMD, no collectives):
  Launch 1 — attention, tensor-parallel over heads (2 heads/core). Host
    pre-computes hT = (rmsnorm(x) )^T (the norm scale depends only on x) and
    folds g1 and 1/sqrt(dh) into the per-core head weight slices. Each core
    computes Q^T/K^T/V^T projections, a causal streaming softmax (no max
    subtraction — scores are O(1) for this problem), and the per-head
    unnormalized attention output o_u^T plus softmax denominators, all in
    transposed [feature, seq] space so no on-chip activation transposes are
    needed (V is transposed on the PE via identity matmuls).
  Host — concatenates per-head o_u^T, divides by denominators.
  Launch 2 — o@WO + residual, rmsnorm2, MLP, residual; sequence-parallel
    (512 tokens/core), also fully in transposed space. Host transposes the
    assembled output back.

Matmuls run in float32r (full PE rate at moving-dim 512; ~2e-4 relative
rounding); everything else is f32.
"""

import sys
import types

import numpy as np


# ---------------------------------------------------------------------------
# Environment compatibility shims (inlined — kernel.py must be self-contained)
# ---------------------------------------------------------------------------
def _install_compat():
    try:
        import trn_agent_boot.trn_boot as _tb

        if "antenv.axon_hooks" not in sys.modules:
            _hook = _tb._ntff_profile_via_ctypes("/opt/axon/libaxon_pjrt.so")
            _m = types.ModuleType("antenv.axon_hooks")
            _m.get_axon_ntff_profile_hook = lambda: _hook
            sys.modules["antenv.axon_hooks"] = _m
    except Exception:
        pass

    import concourse.mybir as mybir
    from concourse import tile as _tile
    from concourse import bass_utils as _bass_utils
    from concourse.vector_clock import ScopedClock as _ScopedClock

    _bass_utils.upload_artifacts = lambda tmpdir: f"local:{tmpdir}"

    def _patched_drain_and_barrier(self, tick_clock, wait_clock):
        nc = self.nc
        drain_inst = nc.sync.drain()
        wait_clock.add_sem_waits(
            drain_inst.ins, _ScopedClock({None: tick_clock.global_clock})
        )
        si = drain_inst.ins.sync_info
        waits = list(si.on_wait or []) if si else []
        if len(waits) > 1:
            drain_inst.ins.sync_info = mybir.SyncInfo(
                on_wait=waits[:1], on_update=list(si.on_update or [])
            )
            for i in range(1, len(waits)):
                nop = nc.sync.nop(nofuse=True, hint="drain_wait_split")
                nop.ins.sync_info = mybir.SyncInfo(on_wait=waits[i : i + 1], on_update=[])
        nc.all_engine_barrier()
        assert self.sems is not None
        popped = nc._tile_sem_poison_stack.pop()
        assert popped is self._sem_poison
        nc.clear_and_free_semaphores(list(self.sems.allocated().values()))
        nc.all_engine_barrier()

    _tile.TileContext._drain_and_barrier = _patched_drain_and_barrier


_install_compat()

import concourse.bass as bass
import concourse.mybir as mybir
from concourse import tile
from concourse.masks import make_identity
from concourse.bass_utils import run_bass_kernel_spmd

F32 = mybir.dt.float32
F32R = mybir.dt.float32r
AF = mybir.ActivationFunctionType
ALU = mybir.AluOpType
AX = mybir.AxisListType

S, D, H, DH = 4096, 1024, 16, 64
NCORES = 8
HPC = H // NCORES          # heads per core = 2
SC = S // NCORES           # seq chunk per core = 512
NCH = S // SC              # number of 512-chunks = 8
NDT = D // 128             # d-tiles = 8
EPS = 1e-6


def _split_multi_waits(nc, max_waits: int = 1):
    """This walrus build accepts only one sem-wait per instruction; hoist
    extras onto fresh NoOps inserted just before, on the same engine."""
    n_split = 0
    for fn in nc.m.functions:
        for blk in fn.blocks:
            out = []
            changed = False
            for inst in blk.instructions:
                si = inst.sync_info
                waits = list(si.on_wait or []) if si else []
                if len(waits) > max_waits:
                    changed = True
                    for i in range(0, len(waits) - max_waits, max_waits):
                        nop = mybir.InstNoOp(
                            name=f"I-waitsplit-{n_split}", ins=[], outs=[]
                        )
                        n_split += 1
                        nop.engine = inst.engine
                        nop.sync_info = mybir.SyncInfo(
                            on_wait=waits[i : i + max_waits], on_update=[]
                        )
                        out.append(nop)
                    inst.sync_info = mybir.SyncInfo(
                        on_wait=waits[len(waits) - max_waits :],
                        on_update=list(si.on_update or []),
                    )
                out.append(inst)
            if changed:
                blk.instructions = out
    return n_split


# ---------------------------------------------------------------------------
# Launch 1: head-sharded attention
# ---------------------------------------------------------------------------
def build_l1():
    nc = bass.Bass("TRN2", target_bir_lowering=False, debug=False)
    ht = nc.declare_dram_parameter("ht", [D, S], F32R, isOutput=False)
    wq = nc.declare_dram_parameter("wq", [D, 128], F32R, isOutput=False)
    wk = nc.declare_dram_parameter("wk", [D, 128], F32R, isOutput=False)
    wv = nc.declare_dram_parameter("wv", [D, 128], F32R, isOutput=False)
    oden = nc.declare_dram_parameter("oden", [HPC, DH + 1, S], F32, isOutput=True)

    with tile.TileContext(nc) as tc:
        with (
            tc.tile_pool(name="const", bufs=1) as const,
            tc.tile_pool(name="wsb", bufs=1) as wsb,
            tc.tile_pool(name="hsb", bufs=3) as hsb,
            tc.tile_pool(name="qt", bufs=NCH) as qt_pool,
            tc.tile_pool(name="kt", bufs=NCH) as kt_pool,
            tc.tile_pool(name="vc", bufs=NCH) as vc_pool,
            tc.tile_pool(name="vt", bufs=2) as vt_pool,
            tc.tile_pool(name="pp", bufs=4) as p_pool,
            tc.tile_pool(name="stg", bufs=2) as stg_pool,
        ):
            # constants
            # identity replicated in both partition halves so head-1 transposes
            # (base_partition 64) can use a matching-base identity slice
            ident = const.tile([128, 64], F32R)
            idf = const.tile([128, 64], F32)
            make_identity(nc, idf[0:64, :])
            make_identity(nc, idf[64:128, :])
            nc.vector.tensor_copy(ident[:], idf[:])
            ones_col = const.tile([128, 1], F32)
            nc.vector.memset(ones_col[:], 1.0)
            onesr_col = const.tile([128, 1], F32R)
            nc.vector.tensor_copy(onesr_col[:], ones_col[:])

            # weights: [128 (d-slice), NDT*128] blocks
            wq_sb = wsb.tile([128, NDT * 128], F32R)
            wk_sb = wsb.tile([128, NDT * 128], F32R)
            wv_sb = wsb.tile([128, NDT * 128], F32R)
            for dt in range(NDT):
                nc.sync.dma_start(
                    out=wq_sb[:, dt * 128 : (dt + 1) * 128],
                    in_=wq[dt * 128 : (dt + 1) * 128, :],
                )
                nc.sync.dma_start(
                    out=wk_sb[:, dt * 128 : (dt + 1) * 128],
                    in_=wk[dt * 128 : (dt + 1) * 128, :],
                )
                nc.sync.dma_start(
                    out=wv_sb[:, dt * 128 : (dt + 1) * 128],
                    in_=wv[dt * 128 : (dt + 1) * 128, :],
                )

            qt_tiles, kt_tiles, vc_tiles = [], [], []
            # ---- phase A (all chunks): dense PE block — load hT chunk,
            #      project Q^T/K^T/V^T, PE-transpose V into natural layout.
            with tc.tile_pool(name="psp", bufs=3, space="PSUM") as ps_proj:
                for qc in range(NCH):
                    h_c = hsb.tile([128, NDT * SC], F32R, tag="hsb")
                    for dt in range(NDT):
                        nc.sync.dma_start(
                            out=h_c[:, dt * SC : (dt + 1) * SC],
                            in_=ht[dt * 128 : (dt + 1) * 128, qc * SC : (qc + 1) * SC],
                        )

                    q_c = qt_pool.tile([128, SC], F32R, tag="qt")
                    k_c = kt_pool.tile([128, SC], F32R, tag="kt")
                    v_c = vc_pool.tile([128, 4 * 2 * (DH + 1)], F32R, tag="vc")
                    qt_tiles.append(q_c)
                    kt_tiles.append(k_c)
                    vc_tiles.append(v_c)

                    for w_sb, dst in ((wq_sb, q_c), (wk_sb, k_c)):
                        ps = ps_proj.tile([128, SC], F32, tag="psp")
                        for dt in range(NDT):
                            nc.tensor.matmul(
                                ps[:],
                                w_sb[:, dt * 128 : (dt + 1) * 128],
                                h_c[:, dt * SC : (dt + 1) * SC],
                                start=(dt == 0),
                                stop=(dt == NDT - 1),
                            )
                        nc.scalar.copy(dst[:], ps[:])

                    # V^T then PE-transpose into natural V layout (+ones col)
                    ps = ps_proj.tile([128, SC], F32, tag="psp")
                    for dt in range(NDT):
                        nc.tensor.matmul(
                            ps[:],
                            wv_sb[:, dt * 128 : (dt + 1) * 128],
                            h_c[:, dt * SC : (dt + 1) * SC],
                            start=(dt == 0),
                            stop=(dt == NDT - 1),
                        )
                    vt_c = vt_pool.tile([128, SC], F32R, tag="vt")
                    nc.vector.tensor_copy(vt_c[:], ps[:])
                    for st in range(4):
                        for hh in range(HPC):
                            pst = ps_proj.tile([128, 64], F32R, tag="psp")
                            nc.tensor.transpose(
                                pst[:],
                                vt_c[hh * 64 : (hh + 1) * 64, st * 128 : (st + 1) * 128],
                                ident[hh * 64 : (hh + 1) * 64, :],
                            )
                            base = st * 2 * (DH + 1) + hh * (DH + 1)
                            nc.vector.tensor_copy(
                                v_c[:, base : base + DH], pst[:]
                            )
                            nc.vector.tensor_copy(
                                v_c[:, base + DH : base + DH + 1], ones_col[:]
                            )

            # ---- phase B (all chunks): causal attention
            with (
                tc.tile_pool(name="pss", bufs=2, space="PSUM") as ps_scores,
                tc.tile_pool(name="psa", bufs=2, space="PSUM") as ps_av,
            ):
                GK = 3  # k-tiles per exp batch ([128, 1536] PSUM, 3 banks)
                for qc in range(NCH):
                    nkt = 4 * (qc + 1)          # causal k-tiles of 128
                    o_ps = [
                        ps_av.tile([DH + 1, SC], F32, tag="psa", name=f"o_ps_{qc}_{hh}")
                        for hh in range(HPC)
                    ]
                    groups = [
                        list(range(g0, min(g0 + GK, nkt))) for g0 in range(0, nkt, GK)
                    ]
                    for gi, kts in enumerate(groups):
                        w = len(kts) * SC
                        s_ps = [None, None]
                        for hh in range(HPC):
                            s_ps[hh] = ps_scores.tile(
                                [128, GK * SC], F32, tag="pss",
                                name=f"s_ps_{qc}_{gi}_{hh}",
                            )
                            for j, kt in enumerate(kts):
                                c, t = kt // 4, kt % 4
                                nc.tensor.matmul(
                                    s_ps[hh][:, j * SC : (j + 1) * SC],
                                    kt_tiles[c][
                                        hh * 64 : (hh + 1) * 64, t * 128 : (t + 1) * 128
                                    ],
                                    qt_tiles[qc][hh * 64 : (hh + 1) * 64, :],
                                    start=True,
                                    stop=True,
                                )
                        for hh in range(HPC):
                            p_t = p_pool.tile([128, GK * SC], F32R, tag="pp")
                            nc.scalar.activation(p_t[:, :w], s_ps[hh][:, :w], AF.Exp)
                            for j, kt in enumerate(kts):
                                if kt >= 4 * qc:  # diagonal tile: zero k > q
                                    nc.gpsimd.affine_select(
                                        out=p_t[:, j * SC : (j + 1) * SC],
                                        in_=p_t[:, j * SC : (j + 1) * SC],
                                        compare_op=ALU.is_ge,
                                        fill=0.0,
                                        base=-128 * (kt - 4 * qc),
                                        pattern=[[1, SC]],
                                        channel_multiplier=-1,
                                    )
                            for j, kt in enumerate(kts):
                                c, t = kt // 4, kt % 4
                                base = t * 2 * (DH + 1) + hh * (DH + 1)
                                nc.tensor.matmul(
                                    o_ps[hh][:],
                                    vc_tiles[c][:, base : base + DH + 1],
                                    p_t[:, j * SC : (j + 1) * SC],
                                    start=(kt == 0),
                                    stop=(kt == nkt - 1),
                                )
                    for hh in range(HPC):
                        stg = stg_pool.tile([DH + 1, SC], F32, tag="stg")
                        nc.vector.tensor_copy(stg[:], o_ps[hh][:])
                        nc.sync.dma_start(
                            out=oden[hh, :, qc * SC : (qc + 1) * SC], in_=stg[:]
                        )

    _split_multi_waits(nc)
    return nc


# ---------------------------------------------------------------------------
# Launch 2: sequence-sharded  WO + residual + rmsnorm + MLP + residual
# ---------------------------------------------------------------------------
def build_l2():
    nc = bass.Bass("TRN2", target_bir_lowering=False, debug=False)
    xt = nc.declare_dram_parameter("xt", [D, SC], F32, isOutput=False)
    ot = nc.declare_dram_parameter("ot", [D, SC], F32R, isOutput=False)
    wo = nc.declare_dram_parameter("wo", [D, D], F32R, isOutput=False)
    w1 = nc.declare_dram_parameter("w1", [D, 4 * D], F32R, isOutput=False)
    w2 = nc.declare_dram_parameter("w2", [4 * D, D], F32R, isOutput=False)
    b1 = nc.declare_dram_parameter("b1", [128, 32], F32, isOutput=False)
    b2 = nc.declare_dram_parameter("b2", [128, 8], F32, isOutput=False)
    yt = nc.declare_dram_parameter("yt", [D, SC], F32, isOutput=True)

    NHT = 4 * D // 128  # 32 hidden tiles

    with tile.TileContext(nc) as tc:
        with (
            tc.tile_pool(name="const", bufs=1) as const,
            tc.tile_pool(name="big", bufs=1) as big,
            tc.tile_pool(name="wt", bufs=6) as wt_pool,
            tc.tile_pool(name="a1", bufs=NHT) as a1_pool,
            tc.tile_pool(name="sq", bufs=2) as sq_pool,
            tc.tile_pool(name="y", bufs=2) as y_pool,
            tc.tile_pool(name="psa", bufs=4, space="PSUM") as ps_a,
            tc.tile_pool(name="psn", bufs=2, space="PSUM") as ps_n,
        ):
            ones_f = const.tile([128, 1], F32)
            nc.vector.memset(ones_f[:], 1.0)
            ones_r = const.tile([128, 1], F32R)
            nc.vector.tensor_copy(ones_r[:], ones_f[:])
            ones_row_f = const.tile([1, 128], F32)
            nc.vector.memset(ones_row_f[:], 1.0)
            ones_row = const.tile([1, 128], F32R)
            nc.vector.tensor_copy(ones_row[:], ones_row_f[:])
            eps_t = const.tile([1, 1], F32)
            nc.vector.memset(eps_t[:], EPS)
            b1_sb = const.tile([128, 32], F32)
            nc.sync.dma_start(out=b1_sb[:], in_=b1[:])
            b2_sb = const.tile([128, 8], F32)
            nc.sync.dma_start(out=b2_sb[:], in_=b2[:])

            xt_sb = big.tile([128, NDT * SC], F32)
            ot_sb = big.tile([128, NDT * SC], F32R)
            xm_sb = big.tile([128, NDT * SC], F32)
            h2_sb = big.tile([128, NDT * SC], F32R)
            for dt in range(NDT):
                nc.sync.dma_start(
                    out=xt_sb[:, dt * SC : (dt + 1) * SC],
                    in_=xt[dt * 128 : (dt + 1) * 128, :],
                )
                nc.sync.dma_start(
                    out=ot_sb[:, dt * SC : (dt + 1) * SC],
                    in_=ot[dt * 128 : (dt + 1) * 128, :],
                )

            # Weight column-slabs: one 512 KB DMA brings [1024, 128] of a
            # weight matrix into SBUF as [128, 8*128] (d-tile-major blocks),
            # alternating between the two HWDGE rings (SP / ACT).
            def slab_dma(idx, w_param, row0, col0):
                w_t = wt_pool.tile([128, NDT * 128], F32R, tag="wt",
                                   name=f"wslab_{idx}_{row0}_{col0}")
                eng = nc.sync if idx % 2 == 0 else nc.scalar
                eng.dma_start(
                    out=w_t[:].rearrange("p (a f) -> p a f", a=NDT),
                    in_=w_param[row0 : row0 + 1024, col0 : col0 + 128].rearrange(
                        "(a p) f -> p a f", p=128
                    ),
                )
                return w_t

            # x_mid^T = WO^T @ o^T + x^T
            for do in range(NDT):
                w_t = slab_dma(do, wo, 0, do * 128)
                ps = ps_a.tile([128, SC], F32, tag="psa")
                for od in range(NDT):
                    nc.tensor.matmul(
                        ps[:],
                        w_t[:, od * 128 : (od + 1) * 128],
                        ot_sb[:, od * SC : (od + 1) * SC],
                        start=(od == 0),
                        stop=(od == NDT - 1),
                    )
                nc.vector.tensor_add(
                    xm_sb[:, do * SC : (do + 1) * SC],
                    ps[:],
                    xt_sb[:, do * SC : (do + 1) * SC],
                )

            # rmsnorm2 (partition reduction via ones-matmul)
            ps_sum = ps_n.tile([1, SC], F32, tag="psn")
            for dt in range(NDT):
                sq = sq_pool.tile([128, SC], F32R, tag="sq")
                nc.vector.tensor_mul(
                    sq[:],
                    xm_sb[:, dt * SC : (dt + 1) * SC],
                    xm_sb[:, dt * SC : (dt + 1) * SC],
                )
                nc.tensor.matmul(
                    ps_sum[:], ones_r[:], sq[:], start=(dt == 0), stop=(dt == NDT - 1)
                )
            rt = sq_pool.tile([1, SC], F32, tag="rt")
            nc.scalar.activation(
                rt[:], ps_sum[:], AF.Sqrt, bias=eps_t[:], scale=1.0 / D
            )
            rr = sq_pool.tile([1, SC], F32R, tag="rt")
            with nc.allow_low_precision(reason="f32r rounding of rmsnorm scale"):
                nc.vector.reciprocal(rr[:], rt[:])
            ps_b = ps_n.tile([128, SC], F32, tag="psn")
            nc.tensor.matmul(ps_b[:], ones_row[:], rr[:], start=True, stop=True)
            for dt in range(NDT):
                nc.vector.tensor_mul(
                    h2_sb[:, dt * SC : (dt + 1) * SC],
                    xm_sb[:, dt * SC : (dt + 1) * SC],
                    ps_b[:],
                )

            # MLP up + relu
            a1_tiles = []
            for ht_i in range(NHT):
                w_t = slab_dma(NDT + ht_i, w1, 0, ht_i * 128)
                ps = ps_a.tile([128, SC], F32, tag="psa")
                for dt in range(NDT):
                    nc.tensor.matmul(
                        ps[:],
                        w_t[:, dt * 128 : (dt + 1) * 128],
                        h2_sb[:, dt * SC : (dt + 1) * SC],
                        start=(dt == 0),
                        stop=(dt == NDT - 1),
                    )
                a1 = a1_pool.tile([128, SC], F32R, tag="a1")
                a1_tiles.append(a1)
                nc.scalar.activation(
                    a1[:], ps[:], AF.Relu, bias=b1_sb[:, ht_i : ht_i + 1], scale=1.0
                )

            # MLP down + bias + residual (W2 column consumed as 4 row-sub-slabs)
            for do in range(NDT):
                ps = ps_a.tile([128, SC], F32, tag="psa")
                for s in range(4):
                    w_t = slab_dma(NDT + NHT + do * 4 + s, w2, s * 1024, do * 128)
                    for a in range(NDT):
                        ht_i = s * NDT + a
                        nc.tensor.matmul(
                            ps[:],
                            w_t[:, a * 128 : (a + 1) * 128],
                            a1_tiles[ht_i][:],
                            start=(ht_i == 0),
                            stop=(ht_i == NHT - 1),
                        )
                y = y_pool.tile([128, SC], F32, tag="y")
                nc.vector.tensor_add(y[:], ps[:], xm_sb[:, do * SC : (do + 1) * SC])
                nc.vector.tensor_scalar_add(y[:], y[:], b2_sb[:, do : do + 1])
                nc.sync.dma_start(out=yt[do * 128 : (do + 1) * 128, :], in_=y[:])

    _split_multi_waits(nc)
    return nc


_NC_L1 = None
_NC_L2 = None


def _get_programs():
    global _NC_L1, _NC_L2
    if _NC_L1 is None:
        _NC_L1 = build_l1()
        _NC_L2 = build_l2()
    return _NC_L1, _NC_L2


def _prep_l1(x, g1, WQ, WK, WV):
    ms = (x * x).mean(-1)
    r1 = 1.0 / np.sqrt(ms + EPS)
    ht = np.ascontiguousarray((x * r1[:, None]).T)
    fq = (g1[None, :, None] * WQ) / np.sqrt(DH)
    fk = g1[None, :, None] * WK
    fv = g1[None, :, None] * WV
    in_maps = []
    for i in range(NCORES):
        in_maps.append(
            {
                "ht": ht,
                "wq": np.ascontiguousarray(
                    np.concatenate([fq[2 * i], fq[2 * i + 1]], axis=1)
                ),
                "wk": np.ascontiguousarray(
                    np.concatenate([fk[2 * i], fk[2 * i + 1]], axis=1)
                ),
                "wv": np.ascontiguousarray(
                    np.concatenate([fv[2 * i], fv[2 * i + 1]], axis=1)
                ),
            }
        )
    return in_maps


def _prep_l2(x, oden_results, g2, WO, W1, B1, W2, B2):
    oT = np.empty((D, S), np.float32)
    for i in range(NCORES):
        od = oden_results[i]["oden"]
        for hh in range(HPC):
            g = 2 * i + hh
            oT[g * DH : (g + 1) * DH] = od[hh, :DH] / od[hh, DH : DH + 1]
    xT = x.T
    w1 = np.ascontiguousarray(g2[:, None] * W1)
    b1r = np.ascontiguousarray(B1.reshape(32, 128).T)
    b2r = np.ascontiguousarray(B2.reshape(8, 128).T)
    in_maps = []
    for i in range(NCORES):
        sl = slice(i * SC, (i + 1) * SC)
        in_maps.append(
            {
                "xt": np.ascontiguousarray(xT[:, sl]),
                "ot": np.ascontiguousarray(oT[:, sl]),
                "wo": WO,
                "w1": w1,
                "w2": W2,
                "b1": b1r,
                "b2": b2r,
            }
        )
    return in_maps


def kernel(**inputs):
    x = np.asarray(inputs["x"], dtype=np.float32)[0]
    g1 = np.asarray(inputs["g1"], dtype=np.float32)
    g2 = np.asarray(inputs["g2"], dtype=np.float32)
    WQ = np.asarray(inputs["WQ"], dtype=np.float32)
    WK = np.asarray(inputs["WK"], dtype=np.float32)
    WV = np.asarray(inputs["WV"], dtype=np.float32)
    WO = np.asarray(inputs["WO"], dtype=np.float32)
    W1 = np.asarray(inputs["W1"], dtype=np.float32)
    B1 = np.asarray(inputs["B1"], dtype=np.float32)
    W2 = np.asarray(inputs["W2"], dtype=np.float32)
    B2 = np.asarray(inputs["B2"], dtype=np.float32)

    nc1, nc2 = _get_programs()
    core_ids = list(range(NCORES))

    in1 = _prep_l1(x, g1, WQ, WK, WV)
    res1 = run_bass_kernel_spmd(nc1, in1, core_ids).results

    in2 = _prep_l2(x, res1, g2, WO, W1, B1, W2, B2)
    res2 = run_bass_kernel_spmd(nc2, in2, core_ids).results

    yT = np.concatenate([res2[i]["yt"] for i in range(NCORES)], axis=1)
    return np.ascontiguousarray(yT.T).reshape(1, S, D)



# revision 9
# speedup vs baseline: 1.6008x; 1.6008x over previous
"""Trainium2 Bass kernel for nn_DecoderBlock (B=1, S=4096, D=1024, H=16, dh=64).

Strategy (8 NeuronCores, SPMD, no collectives):
  Launch 1 — attention, tensor-parallel over heads (2 heads/core). Host
    pre-computes hT = rmsnorm(x)*g1 transposed (the norm scale depends only
    on x) and folds 1/sqrt(dh) into the per-core WQ head slices. Each core
    computes Q^T/K^T/V^T projections, a causal streaming softmax (no max
    subtraction — scores are O(1) for this problem), and the per-head
    unnormalized attention output o_u^T plus softmax denominators, all in
    transposed [feature, seq] space (V is transposed on the PE via identity
    matmuls).

    The instruction stream is software-pipelined so the PE never idles
    (keeps the HAM clock gate at 2.4 GHz): attention-value matmuls lag one
    k-tile group behind the score matmuls (covering the exp latency on the
    ACT engine), and the Q/K/V projections of later chunks are drip-fed
    into the attention stream as PE filler work. Causal masking of the
    diagonal tiles is a DVE multiply with 4 static mask tiles.
  Host — concatenates per-head o_u^T, divides by denominators.
  Launch 2 — o@WO + residual, rmsnorm2, MLP, residual; sequence-parallel
    (512 tokens/core). Weights are streamed as bf16, host-prepacked into
    partition-contiguous SBUF layout (2KB+ DMA lines). RMSNorm partial sums
    are interleaved with the WO matmuls to shrink the serial bubble.

Matmuls run in bf16/f32r (1 cycle/row at moving-dim 512); accumulation is
f32 in PSUM. Residual paths (x, x_mid, y) stay f32.
"""

import sys
import types
from collections import deque

import numpy as np
import ml_dtypes

BF16NP = ml_dtypes.bfloat16


# ---------------------------------------------------------------------------
# Environment compatibility shims (inlined — kernel.py must be self-contained)
# ---------------------------------------------------------------------------
def _install_compat():
    try:
        import trn_agent_boot.trn_boot as _tb

        if "antenv.axon_hooks" not in sys.modules:
            _hook = _tb._ntff_profile_via_ctypes("/opt/axon/libaxon_pjrt.so")
            _m = types.ModuleType("antenv.axon_hooks")
            _m.get_axon_ntff_profile_hook = lambda: _hook
            sys.modules["antenv.axon_hooks"] = _m
    except Exception:
        pass

    import concourse.mybir as mybir
    from concourse import tile as _tile
    from concourse import bass_utils as _bass_utils
    from concourse.vector_clock import ScopedClock as _ScopedClock

    _bass_utils.upload_artifacts = lambda tmpdir: f"local:{tmpdir}"

    def _patched_drain_and_barrier(self, tick_clock, wait_clock):
        nc = self.nc
        drain_inst = nc.sync.drain()
        wait_clock.add_sem_waits(
            drain_inst.ins, _ScopedClock({None: tick_clock.global_clock})
        )
        si = drain_inst.ins.sync_info
        waits = list(si.on_wait or []) if si else []
        if len(waits) > 1:
            drain_inst.ins.sync_info = mybir.SyncInfo(
                on_wait=waits[:1], on_update=list(si.on_update or [])
            )
            for i in range(1, len(waits)):
                nop = nc.sync.nop(nofuse=True, hint="drain_wait_split")
                nop.ins.sync_info = mybir.SyncInfo(on_wait=waits[i : i + 1], on_update=[])
        nc.all_engine_barrier()
        assert self.sems is not None
        popped = nc._tile_sem_poison_stack.pop()
        assert popped is self._sem_poison
        nc.clear_and_free_semaphores(list(self.sems.allocated().values()))
        nc.all_engine_barrier()

    _tile.TileContext._drain_and_barrier = _patched_drain_and_barrier


_install_compat()

import concourse.bass as bass
import concourse.mybir as mybir
from concourse import tile
from concourse.masks import make_identity
from concourse.bass_utils import run_bass_kernel_spmd

F32 = mybir.dt.float32
F32R = mybir.dt.float32r
BF16 = mybir.dt.bfloat16
AF = mybir.ActivationFunctionType
ALU = mybir.AluOpType

S, D, H, DH = 4096, 1024, 16, 64
NCORES = 8
HPC = H // NCORES          # heads per core = 2
SC = S // NCORES           # seq chunk per core = 512
NCH = S // SC              # number of 512-chunks = 8
NDT = D // 128             # d-tiles = 8
NHT = 4 * D // 128         # hidden tiles = 32
EPS = 1e-6

# chunks projected up-front before the attention stream starts; the rest are
# drip-fed as PE filler between attention groups
UPFRONT_CHUNKS = 4


def _split_multi_waits(nc, max_waits: int = 1):
    """This walrus build accepts only one sem-wait per instruction; hoist
    extras onto fresh NoOps inserted just before, on the same engine."""
    n_split = 0
    for fn in nc.m.functions:
        for blk in fn.blocks:
            out = []
            changed = False
            for inst in blk.instructions:
                si = inst.sync_info
                waits = list(si.on_wait or []) if si else []
                if len(waits) > max_waits:
                    changed = True
                    for i in range(0, len(waits) - max_waits, max_waits):
                        nop = mybir.InstNoOp(
                            name=f"I-waitsplit-{n_split}", ins=[], outs=[]
                        )
                        n_split += 1
                        nop.engine = inst.engine
                        nop.sync_info = mybir.SyncInfo(
                            on_wait=waits[i : i + max_waits], on_update=[]
                        )
                        out.append(nop)
                    inst.sync_info = mybir.SyncInfo(
                        on_wait=waits[len(waits) - max_waits :],
                        on_update=list(si.on_update or []),
                    )
                out.append(inst)
            if changed:
                blk.instructions = out
    return n_split


# ---------------------------------------------------------------------------
# Launch 1: head-sharded attention
# ---------------------------------------------------------------------------
def build_l1():
    nc = bass.Bass("TRN2", target_bir_lowering=False, debug=False)
    ht = nc.declare_dram_parameter("ht", [128, NCH, NDT * SC], BF16, isOutput=False)
    wq = nc.declare_dram_parameter("wq", [128, NDT * 128], BF16, isOutput=False)
    wk = nc.declare_dram_parameter("wk", [128, NDT * 128], BF16, isOutput=False)
    wv = nc.declare_dram_parameter("wv", [128, NDT * 128], BF16, isOutput=False)
    msk = nc.declare_dram_parameter("msk", [128, 4 * SC], BF16, isOutput=False)
    oden = nc.declare_dram_parameter("oden", [HPC, DH + 1, S], F32, isOutput=True)

    with tile.TileContext(nc) as tc:
        with (
            tc.tile_pool(name="const", bufs=1) as const,
            tc.tile_pool(name="hsb", bufs=3) as hsb,
            tc.tile_pool(name="qt", bufs=NCH) as qt_pool,
            tc.tile_pool(name="kt", bufs=NCH) as kt_pool,
            tc.tile_pool(name="vc", bufs=NCH) as vc_pool,
            tc.tile_pool(name="vt", bufs=2) as vt_pool,
            tc.tile_pool(name="pp", bufs=3) as p_pool,
            tc.tile_pool(name="stg", bufs=2) as stg_pool,
            tc.tile_pool(name="psp", bufs=2, space="PSUM") as ps_proj,
            tc.tile_pool(name="pss", bufs=2, space="PSUM") as ps_scores,
            tc.tile_pool(name="psa", bufs=2, space="PSUM") as ps_av,
        ):
            # constants: identity replicated in both partition halves so
            # head-1 transposes (base_partition 64) can use a matching slice
            idf = const.tile([128, 64], F32)
            make_identity(nc, idf[0:64, :])
            make_identity(nc, idf[64:128, :])
            ident = const.tile([128, 64], BF16)
            nc.vector.tensor_copy(ident[:], idf[:])

            # weights + masks on the ACT HWDGE ring so they land in parallel
            # with the first hT chunk DMAs on the SP ring
            wq_sb = const.tile([128, NDT * 128], BF16)
            wk_sb = const.tile([128, NDT * 128], BF16)
            wv_sb = const.tile([128, NDT * 128], BF16)
            nc.scalar.dma_start(out=wq_sb[:], in_=wq[:])
            nc.scalar.dma_start(out=wk_sb[:], in_=wk[:])
            nc.scalar.dma_start(out=wv_sb[:], in_=wv[:])
            msk_sb = const.tile([128, 4 * SC], BF16)
            nc.scalar.dma_start(out=msk_sb[:], in_=msk[:])

            qt_tiles = [None] * NCH
            kt_tiles = [None] * NCH
            vc_tiles = [None] * NCH

            # Q/K/V projection for one chunk, as a generator yielding after
            # every PE instruction so it can be drip-fed as filler.
            def proj_gen(c):
                h_c = hsb.tile([128, NDT * SC], BF16, tag="hsb")
                nc.sync.dma_start(out=h_c[:], in_=ht[:, c, :])

                q_c = qt_pool.tile([128, SC], BF16, tag="qt")
                k_c = kt_pool.tile([128, SC], BF16, tag="kt")
                qt_tiles[c] = q_c
                kt_tiles[c] = k_c

                for w_sb, dst in ((wq_sb, q_c), (wk_sb, k_c)):
                    ps = ps_proj.tile([128, SC], F32, tag="psp")
                    for dt in range(NDT):
                        nc.tensor.matmul(
                            ps[:],
                            w_sb[:, dt * 128 : (dt + 1) * 128],
                            h_c[:, dt * SC : (dt + 1) * SC],
                            start=(dt == 0),
                            stop=(dt == NDT - 1),
                        )
                        yield
                    nc.vector.tensor_copy(dst[:], ps[:])

                ps = ps_proj.tile([128, SC], F32, tag="psp")
                for dt in range(NDT):
                    nc.tensor.matmul(
                        ps[:],
                        wv_sb[:, dt * 128 : (dt + 1) * 128],
                        h_c[:, dt * SC : (dt + 1) * SC],
                        start=(dt == 0),
                        stop=(dt == NDT - 1),
                    )
                    yield
                vt_c = vt_pool.tile([128, SC], BF16, tag="vt")
                nc.vector.tensor_copy(vt_c[:], ps[:])

                # PE-transpose V into natural [kpos, dh] layout (+ones col)
                v_c = vc_pool.tile([128, 4 * HPC * (DH + 1)], BF16, tag="vc")
                vc_tiles[c] = v_c
                v_c3 = v_c[:].rearrange("p (a f) -> p a f", a=4 * HPC)
                nc.vector.memset(v_c3[:, :, DH : DH + 1], 1.0)
                for st in range(4):
                    for hh in range(HPC):
                        pst = ps_proj.tile([128, 64], BF16, tag="psp")
                        nc.tensor.transpose(
                            pst[:],
                            vt_c[hh * 64 : (hh + 1) * 64, st * 128 : (st + 1) * 128],
                            ident[hh * 64 : (hh + 1) * 64, :],
                        )
                        base = (st * HPC + hh) * (DH + 1)
                        nc.vector.tensor_copy(v_c[:, base : base + DH], pst[:])
                        yield

            gens = deque(proj_gen(c) for c in range(NCH))

            def run_filler(n):
                done = 0
                while done < n and gens:
                    try:
                        next(gens[0])
                        done += 1
                    except StopIteration:
                        gens.popleft()
                return done

            steps_left = NCH * (3 * NDT + 4 * HPC)  # PE steps in all gens

            # up-front projections for the first chunks
            while len(gens) > NCH - UPFRONT_CHUNKS:
                steps_left -= run_filler(1)

            groups_left = sum(4 * (qc + 1) for qc in range(NCH))

            def emit_av(qc, kt, p_t, o_ps):
                c, t = kt // 4, kt % 4
                nktq = 4 * (qc + 1)
                for hh in range(HPC):
                    base = (t * HPC + hh) * (DH + 1)
                    nc.tensor.matmul(
                        o_ps[hh][:],
                        vc_tiles[c][:, base : base + DH + 1],
                        p_t[:, hh * SC : (hh + 1) * SC],
                        start=(kt == 0),
                        stop=(kt == nktq - 1),
                    )
                if kt == nktq - 1:
                    for hh in range(HPC):
                        stg = stg_pool.tile([DH + 1, SC], F32, tag="stg")
                        nc.vector.tensor_copy(stg[:], o_ps[hh][:])
                        nc.sync.dma_start(
                            out=oden[hh, :, qc * SC : (qc + 1) * SC], in_=stg[:]
                        )

            pending = None  # (qc, kt, p_t, o_ps)
            for qc in range(NCH):
                o_ps = [
                    ps_av.tile([DH + 1, SC], F32, tag="psa", name=f"o_ps_{qc}_{hh}")
                    for hh in range(HPC)
                ]
                nkt = 4 * (qc + 1)
                for kt in range(nkt):
                    c, t = kt // 4, kt % 4
                    s_ps = ps_scores.tile(
                        [128, HPC * SC], F32, tag="pss", name=f"s_ps_{qc}_{kt}"
                    )
                    for hh in range(HPC):
                        nc.tensor.matmul(
                            s_ps[:, hh * SC : (hh + 1) * SC],
                            kt_tiles[c][
                                hh * 64 : (hh + 1) * 64, t * 128 : (t + 1) * 128
                            ],
                            qt_tiles[qc][hh * 64 : (hh + 1) * 64, :],
                            start=True,
                            stop=True,
                        )
                    p_t = p_pool.tile([128, HPC * SC], BF16, tag="pp")
                    nc.scalar.activation(p_t[:], s_ps[:], AF.Exp)
                    if kt >= 4 * qc:  # diagonal tile: zero k > q
                        for hh in range(HPC):
                            nc.vector.tensor_mul(
                                p_t[:, hh * SC : (hh + 1) * SC],
                                p_t[:, hh * SC : (hh + 1) * SC],
                                msk_sb[:, t * SC : (t + 1) * SC],
                            )
                    # drip-feed projection filler to keep the PE gap-free
                    want = 2 if steps_left > groups_left else 1
                    steps_left -= run_filler(want)
                    groups_left -= 1
                    if pending is not None:
                        emit_av(*pending)
                    pending = (qc, kt, p_t, o_ps)
            emit_av(*pending)

    _split_multi_waits(nc)
    return nc


# ---------------------------------------------------------------------------
# Launch 2: sequence-sharded  WO + residual + rmsnorm + MLP + residual
# ---------------------------------------------------------------------------
def build_l2():
    nc = bass.Bass("TRN2", target_bir_lowering=False, debug=False)
    xt = nc.declare_dram_parameter("xt", [128, NDT * SC], F32, isOutput=False)
    ot = nc.declare_dram_parameter("ot", [128, NDT * SC], BF16, isOutput=False)
    wo = nc.declare_dram_parameter("wo", [128, NDT * NDT * 128], BF16, isOutput=False)
    w1 = nc.declare_dram_parameter("w1", [128, NHT, NDT * 128], BF16, isOutput=False)
    w2 = nc.declare_dram_parameter("w2", [128, NDT, NHT * 128], BF16, isOutput=False)
    b1 = nc.declare_dram_parameter("b1", [128, NHT], F32, isOutput=False)
    b2 = nc.declare_dram_parameter("b2", [128, NDT], F32, isOutput=False)
    yt = nc.declare_dram_parameter("yt", [128, NDT * SC], F32, isOutput=True)

    with tile.TileContext(nc) as tc:
        with (
            tc.tile_pool(name="const", bufs=1) as const,
            tc.tile_pool(name="big", bufs=1) as big,
            tc.tile_pool(name="wt", bufs=4) as wt_pool,
            tc.tile_pool(name="a1", bufs=NHT) as a1_pool,
            tc.tile_pool(name="sq", bufs=2) as sq_pool,
            tc.tile_pool(name="y", bufs=2) as y_pool,
            tc.tile_pool(name="psa", bufs=4, space="PSUM") as ps_a,
            tc.tile_pool(name="psn", bufs=2, space="PSUM") as ps_n,
        ):
            ones_f = const.tile([128, 1], F32)
            nc.vector.memset(ones_f[:], 1.0)
            ones_r = const.tile([128, 1], F32R)
            nc.vector.tensor_copy(ones_r[:], ones_f[:])
            ones_row_f = const.tile([1, 128], F32)
            nc.vector.memset(ones_row_f[:], 1.0)
            ones_row = const.tile([1, 128], F32R)
            nc.vector.tensor_copy(ones_row[:], ones_row_f[:])
            eps_t = const.tile([1, 1], F32)
            nc.vector.memset(eps_t[:], EPS)
            warm_t = const.tile([1, 1], F32)
            # pre-load the Sqrt activation table while ACT is otherwise idle
            nc.scalar.activation(warm_t[:], eps_t[:], AF.Sqrt)
            b1_sb = const.tile([128, NHT], F32)
            b2_sb = const.tile([128, NDT], F32)

            xt_sb = big.tile([128, NDT * SC], F32)
            ot_sb = big.tile([128, NDT * SC], BF16)
            wo_sb = big.tile([128, NDT * NDT * 128], BF16)
            xm_sb = big.tile([128, NDT * SC], F32)
            h2_sb = big.tile([128, NDT * SC], BF16)
            # ot first on the SP ring (WO matmuls need it), xt in per-tile
            # slices so the residual adds can start before the full load;
            # WO weights stream per-do on the ACT ring in parallel.
            nc.sync.dma_start(out=ot_sb[:], in_=ot[:])
            for dt in range(NDT):
                nc.sync.dma_start(
                    out=xt_sb[:, dt * SC : (dt + 1) * SC],
                    in_=xt[:, dt * SC : (dt + 1) * SC],
                )
            for do in range(NDT):
                nc.scalar.dma_start(
                    out=wo_sb[:, do * NDT * 128 : (do + 1) * NDT * 128],
                    in_=wo[:, do * NDT * 128 : (do + 1) * NDT * 128],
                )
            nc.scalar.dma_start(out=b1_sb[:], in_=b1[:])
            nc.scalar.dma_start(out=b2_sb[:], in_=b2[:])

            # x_mid^T = WO^T @ o^T + x^T, with rmsnorm partial sums interleaved
            ps_sum = ps_n.tile([1, SC], F32, tag="psn")
            for do in range(NDT):
                ps = ps_a.tile([128, SC], F32, tag="psa")
                for od in range(NDT):
                    nc.tensor.matmul(
                        ps[:],
                        wo_sb[:, (do * NDT + od) * 128 : (do * NDT + od + 1) * 128],
                        ot_sb[:, od * SC : (od + 1) * SC],
                        start=(od == 0),
                        stop=(od == NDT - 1),
                    )
                nc.vector.tensor_add(
                    xm_sb[:, do * SC : (do + 1) * SC],
                    ps[:],
                    xt_sb[:, do * SC : (do + 1) * SC],
                )
                sq = sq_pool.tile([128, SC], F32R, tag="sq")
                nc.scalar.activation(
                    sq[:], xm_sb[:, do * SC : (do + 1) * SC], AF.Square
                )
                nc.tensor.matmul(
                    ps_sum[:], ones_r[:], sq[:], start=(do == 0), stop=(do == NDT - 1)
                )

            # rmsnorm2 scale, broadcast to all partitions via ones matmul
            rt = sq_pool.tile([1, SC], F32, tag="rt")
            nc.scalar.activation(
                rt[:], ps_sum[:], AF.Sqrt, bias=eps_t[:], scale=1.0 / D
            )
            rr = sq_pool.tile([1, SC], F32R, tag="rt")
            with nc.allow_low_precision(reason="f32r rounding of rmsnorm scale"):
                nc.vector.reciprocal(rr[:], rt[:])
            ps_b = ps_n.tile([128, SC], F32, tag="psn")
            nc.tensor.matmul(ps_b[:], ones_row[:], rr[:], start=True, stop=True)
            for dt in range(NDT):
                nc.vector.tensor_mul(
                    h2_sb[:, dt * SC : (dt + 1) * SC],
                    xm_sb[:, dt * SC : (dt + 1) * SC],
                    ps_b[:],
                )

            # MLP up + relu (W1 streamed in 8 slabs of 4 hidden tiles)
            a1_tiles = []
            for g in range(NHT // 4):
                w_t = wt_pool.tile([128, 4 * NDT * 128], BF16, tag="wt")
                nc.sync.dma_start(
                    out=w_t[:].rearrange("p (a f) -> p a f", a=4),
                    in_=w1[:, 4 * g : 4 * (g + 1), :],
                )
                for l in range(4):
                    ht_i = 4 * g + l
                    ps = ps_a.tile([128, SC], F32, tag="psa")
                    for dt in range(NDT):
                        nc.tensor.matmul(
                            ps[:],
                            w_t[
                                :,
                                (l * NDT + dt) * 128 : (l * NDT + dt + 1) * 128,
                            ],
                            h2_sb[:, dt * SC : (dt + 1) * SC],
                            start=(dt == 0),
                            stop=(dt == NDT - 1),
                        )
                    a1 = a1_pool.tile([128, SC], BF16, tag="a1")
                    a1_tiles.append(a1)
                    nc.scalar.activation(
                        a1[:], ps[:], AF.Relu, bias=b1_sb[:, ht_i : ht_i + 1], scale=1.0
                    )

            # MLP down + bias + residual (W2 streamed in 8 slabs)
            for do in range(NDT):
                w_t = wt_pool.tile([128, NHT * 128], BF16, tag="wt")
                nc.sync.dma_start(out=w_t[:], in_=w2[:, do, :])
                ps = ps_a.tile([128, SC], F32, tag="psa")
                for ht_i in range(NHT):
                    nc.tensor.matmul(
                        ps[:],
                        w_t[:, ht_i * 128 : (ht_i + 1) * 128],
                        a1_tiles[ht_i][:],
                        start=(ht_i == 0),
                        stop=(ht_i == NHT - 1),
                    )
                y = y_pool.tile([128, SC], F32, tag="y")
                nc.vector.scalar_tensor_tensor(
                    y[:],
                    ps[:],
                    b2_sb[:, do : do + 1],
                    xm_sb[:, do * SC : (do + 1) * SC],
                    ALU.add,
                    ALU.add,
                )
                nc.scalar.dma_start(out=yt[:, do * SC : (do + 1) * SC], in_=y[:])

    _split_multi_waits(nc)
    return nc


_NC_L1 = None
_NC_L2 = None


def _get_programs():
    global _NC_L1, _NC_L2
    if _NC_L1 is None:
        _NC_L1 = build_l1()
        _NC_L2 = build_l2()
    return _NC_L1, _NC_L2


def _prep_l1(x, g1, WQ, WK, WV):
    ms = (x * x).mean(-1)
    r1 = 1.0 / np.sqrt(ms + EPS)
    h = x * (r1[:, None] * g1[None, :])          # rmsnorm + gain, [S, D]
    hT = np.ascontiguousarray(h.T)               # [D, S]
    # pack [128, chunk, dt*SC]: htp[p, c, dt*SC+j] = hT[dt*128+p, c*SC+j]
    htp = np.ascontiguousarray(
        hT.reshape(NDT, 128, NCH, SC).transpose(1, 2, 0, 3).reshape(128, NCH, NDT * SC)
    ).astype(BF16NP)

    # causal masks for the 4 diagonal k-tile offsets: mask[p, t*SC+j] =
    # (t*128 + p) <= j
    pidx = np.arange(128)[:, None]
    jidx = np.arange(SC)[None, :]
    msk = np.concatenate(
        [((t * 128 + pidx) <= jidx) for t in range(4)], axis=1
    ).astype(BF16NP)

    sq = 1.0 / np.sqrt(DH)
    in_maps = []
    for i in range(NCORES):
        def pack_w(w, scale=1.0):
            # [D, 128] -> [128, dt, 128] partition-major
            return np.ascontiguousarray(
                (w * scale).reshape(NDT, 128, 128).transpose(1, 0, 2).reshape(128, -1)
            ).astype(BF16NP)

        fq = np.concatenate([WQ[2 * i], WQ[2 * i + 1]], axis=1)
        fk = np.concatenate([WK[2 * i], WK[2 * i + 1]], axis=1)
        fv = np.concatenate([WV[2 * i], WV[2 * i + 1]], axis=1)
        in_maps.append(
            {
                "ht": htp,
                "wq": pack_w(fq, sq),
                "wk": pack_w(fk),
                "wv": pack_w(fv),
                "msk": msk,
            }
        )
    return in_maps


def _prep_l2(x, oden_results, g2, WO, W1, B1, W2, B2):
    oT = np.empty((D, S), np.float32)
    for i in range(NCORES):
        od = oden_results[i]["oden"]
        for hh in range(HPC):
            g = 2 * i + hh
            oT[g * DH : (g + 1) * DH] = od[hh, :DH] / od[hh, DH : DH + 1]
    xT = x.T
    w1g = g2[:, None] * W1

    def pack_dmajor(a, rows, cols):
        # [rows*128, cols*128] -> [128, cols, rows, 128]:
        # out[p, c, r, f] = a[r*128+p, c*128+f]
        return np.ascontiguousarray(
            a.reshape(rows, 128, cols, 128).transpose(1, 2, 0, 3)
        ).astype(BF16NP)

    wo_p = pack_dmajor(WO, NDT, NDT).reshape(128, -1)
    w1_p = pack_dmajor(w1g, NDT, NHT).reshape(128, NHT, NDT * 128)
    w2_p = pack_dmajor(W2, NHT, NDT).reshape(128, NDT, NHT * 128)
    b1r = np.ascontiguousarray(B1.reshape(NHT, 128).T)
    b2r = np.ascontiguousarray(B2.reshape(NDT, 128).T)

    in_maps = []
    for i in range(NCORES):
        sl = slice(i * SC, (i + 1) * SC)
        xts = xT[:, sl]   # [D, SC]
        ots = oT[:, sl]
        xtp = np.ascontiguousarray(
            xts.reshape(NDT, 128, SC).transpose(1, 0, 2).reshape(128, -1)
        )
        otp = np.ascontiguousarray(
            ots.reshape(NDT, 128, SC).transpose(1, 0, 2).reshape(128, -1)
        ).astype(BF16NP)
        in_maps.append(
            {
                "xt": xtp,
                "ot": otp,
                "wo": wo_p,
                "w1": w1_p,
                "w2": w2_p,
                "b1": b1r,
                "b2": b2r,
            }
        )
    return in_maps


def kernel(**inputs):
    x = np.asarray(inputs["x"], dtype=np.float32)[0]
    g1 = np.asarray(inputs["g1"], dtype=np.float32)
    g2 = np.asarray(inputs["g2"], dtype=np.float32)
    WQ = np.asarray(inputs["WQ"], dtype=np.float32)
    WK = np.asarray(inputs["WK"], dtype=np.float32)
    WV = np.asarray(inputs["WV"], dtype=np.float32)
    WO = np.asarray(inputs["WO"], dtype=np.float32)
    W1 = np.asarray(inputs["W1"], dtype=np.float32)
    B1 = np.asarray(inputs["B1"], dtype=np.float32)
    W2 = np.asarray(inputs["W2"], dtype=np.float32)
    B2 = np.asarray(inputs["B2"], dtype=np.float32)

    nc1, nc2 = _get_programs()
    core_ids = list(range(NCORES))

    in1 = _prep_l1(x, g1, WQ, WK, WV)
    res1 = run_bass_kernel_spmd(nc1, in1, core_ids).results

    in2 = _prep_l2(x, res1, g2, WO, W1, B1, W2, B2)
    res2 = run_bass_kernel_spmd(nc2, in2, core_ids).results

    yT = np.empty((D, S), np.float32)
    for i in range(NCORES):
        yt = res2[i]["yt"]  # [128, NDT*SC]
        yT[:, i * SC : (i + 1) * SC] = (
            yt.reshape(128, NDT, SC).transpose(1, 0, 2).reshape(D, SC)
        )
    return np.ascontiguousarray(yT.T).reshape(1, S, D)


# revision 16
# speedup vs baseline: 1.6385x; 1.0236x over previous
"""Trainium2 Bass kernel for nn_DecoderBlock (B=1, S=4096, D=1024, H=16, dh=64).

Strategy (8 NeuronCores, SPMD, no collectives):
  Launch 1 — attention, tensor-parallel over heads (2 heads/core). Host
    pre-computes hT = rmsnorm(x)*g1 transposed (the norm scale depends only
    on x) and folds 1/sqrt(dh) into the per-core WQ head slices. Each core
    computes Q^T/K^T/V^T projections, a causal streaming softmax (no max
    subtraction — scores are O(1) for this problem), and the per-head
    unnormalized attention output o_u^T plus softmax denominators, all in
    transposed [feature, seq] space (V is transposed on the PE via identity
    matmuls).

    The instruction stream is software-pipelined so the PE never idles
    (keeps the HAM clock gate at 2.4 GHz): attention-value matmuls lag one
    k-tile group behind the score matmuls (covering the exp latency on the
    ACT engine), and the Q/K/V projections of later chunks are drip-fed
    into the attention stream as PE filler work. Causal masking of the
    diagonal tiles is a DVE multiply with 4 static mask tiles.
  Host — concatenates per-head o_u^T, divides by denominators.
  Launch 2 — o@WO + residual, rmsnorm2, MLP, residual; sequence-parallel
    (512 tokens/core). Weights are streamed as bf16, host-prepacked into
    partition-contiguous SBUF layout (2KB+ DMA lines). RMSNorm partial sums
    are interleaved with the WO matmuls to shrink the serial bubble.

Matmuls run in bf16/f32r (1 cycle/row at moving-dim 512); accumulation is
f32 in PSUM. Residual paths (x, x_mid, y) stay f32.
"""

import sys
import types
from collections import deque

import numpy as np
import ml_dtypes

BF16NP = ml_dtypes.bfloat16


# ---------------------------------------------------------------------------
# Environment compatibility shims (inlined — kernel.py must be self-contained)
# ---------------------------------------------------------------------------
def _install_compat():
    try:
        import trn_agent_boot.trn_boot as _tb

        if "antenv.axon_hooks" not in sys.modules:
            _hook = _tb._ntff_profile_via_ctypes("/opt/axon/libaxon_pjrt.so")
            _m = types.ModuleType("antenv.axon_hooks")
            _m.get_axon_ntff_profile_hook = lambda: _hook
            sys.modules["antenv.axon_hooks"] = _m
    except Exception:
        pass

    import concourse.mybir as mybir
    from concourse import tile as _tile
    from concourse import bass_utils as _bass_utils
    from concourse.vector_clock import ScopedClock as _ScopedClock

    _bass_utils.upload_artifacts = lambda tmpdir: f"local:{tmpdir}"

    def _patched_drain_and_barrier(self, tick_clock, wait_clock):
        nc = self.nc
        drain_inst = nc.sync.drain()
        wait_clock.add_sem_waits(
            drain_inst.ins, _ScopedClock({None: tick_clock.global_clock})
        )
        si = drain_inst.ins.sync_info
        waits = list(si.on_wait or []) if si else []
        if len(waits) > 1:
            drain_inst.ins.sync_info = mybir.SyncInfo(
                on_wait=waits[:1], on_update=list(si.on_update or [])
            )
            for i in range(1, len(waits)):
                nop = nc.sync.nop(nofuse=True, hint="drain_wait_split")
                nop.ins.sync_info = mybir.SyncInfo(on_wait=waits[i : i + 1], on_update=[])
        nc.all_engine_barrier()
        assert self.sems is not None
        popped = nc._tile_sem_poison_stack.pop()
        assert popped is self._sem_poison
        nc.clear_and_free_semaphores(list(self.sems.allocated().values()))
        nc.all_engine_barrier()

    _tile.TileContext._drain_and_barrier = _patched_drain_and_barrier


_install_compat()

import concourse.bass as bass
import concourse.mybir as mybir
from concourse import tile
from concourse.masks import make_identity
from concourse.bass_utils import run_bass_kernel_spmd

F32 = mybir.dt.float32
F32R = mybir.dt.float32r
BF16 = mybir.dt.bfloat16
AF = mybir.ActivationFunctionType
ALU = mybir.AluOpType

S, D, H, DH = 4096, 1024, 16, 64
NCORES = 8
HPC = H // NCORES          # heads per core = 2
SC = S // NCORES           # seq chunk per core = 512
NCH = S // SC              # number of 512-chunks = 8
NDT = D // 128             # d-tiles = 8
NHT = 4 * D // 128         # hidden tiles = 32
EPS = 1e-6

# chunks projected up-front before the attention stream starts; the rest are
# drip-fed as PE filler between attention groups
UPFRONT_CHUNKS = 4


def _split_multi_waits(nc, max_waits: int = 1):
    """This walrus build accepts only one sem-wait per instruction; hoist
    extras onto fresh NoOps inserted just before, on the same engine."""
    n_split = 0
    for fn in nc.m.functions:
        for blk in fn.blocks:
            out = []
            changed = False
            for inst in blk.instructions:
                si = inst.sync_info
                waits = list(si.on_wait or []) if si else []
                if len(waits) > max_waits:
                    changed = True
                    for i in range(0, len(waits) - max_waits, max_waits):
                        nop = mybir.InstNoOp(
                            name=f"I-waitsplit-{n_split}", ins=[], outs=[]
                        )
                        n_split += 1
                        nop.engine = inst.engine
                        nop.sync_info = mybir.SyncInfo(
                            on_wait=waits[i : i + max_waits], on_update=[]
                        )
                        out.append(nop)
                    inst.sync_info = mybir.SyncInfo(
                        on_wait=waits[len(waits) - max_waits :],
                        on_update=list(si.on_update or []),
                    )
                out.append(inst)
            if changed:
                blk.instructions = out
    return n_split


# ---------------------------------------------------------------------------
# Launch 1: head-sharded attention
# ---------------------------------------------------------------------------
def build_l1():
    nc = bass.Bass("TRN2", target_bir_lowering=False, debug=False)
    ht = nc.declare_dram_parameter("ht", [128, NCH, NDT * SC], BF16, isOutput=False)
    wq = nc.declare_dram_parameter("wq", [128, NDT * 128], BF16, isOutput=False)
    wk = nc.declare_dram_parameter("wk", [128, NDT * 128], BF16, isOutput=False)
    wv = nc.declare_dram_parameter("wv", [128, NDT * 128], BF16, isOutput=False)
    msk = nc.declare_dram_parameter("msk", [128, 4 * SC], BF16, isOutput=False)
    oden = nc.declare_dram_parameter("oden", [HPC, DH + 1, S], F32, isOutput=True)

    with tile.TileContext(nc) as tc:
        with (
            tc.tile_pool(name="const", bufs=1) as const,
            tc.tile_pool(name="hsb", bufs=3) as hsb,
            tc.tile_pool(name="qt", bufs=NCH) as qt_pool,
            tc.tile_pool(name="kt", bufs=NCH) as kt_pool,
            tc.tile_pool(name="vc", bufs=NCH) as vc_pool,
            tc.tile_pool(name="vt", bufs=2) as vt_pool,
            tc.tile_pool(name="pp", bufs=3) as p_pool,
            tc.tile_pool(name="stg", bufs=2) as stg_pool,
            tc.tile_pool(name="psp", bufs=2, space="PSUM") as ps_proj,
            tc.tile_pool(name="pss", bufs=2, space="PSUM") as ps_scores,
            tc.tile_pool(name="psa", bufs=2, space="PSUM") as ps_av,
        ):
            # constants: identity replicated in both partition halves so
            # head-1 transposes (base_partition 64) can use a matching slice
            idf = const.tile([128, 64], F32)
            make_identity(nc, idf[0:64, :])
            make_identity(nc, idf[64:128, :])
            ident = const.tile([128, 64], BF16)
            nc.vector.tensor_copy(ident[:], idf[:])

            # weights + masks on the ACT HWDGE ring so they land in parallel
            # with the first hT chunk DMAs on the SP ring
            wq_sb = const.tile([128, NDT * 128], BF16)
            wk_sb = const.tile([128, NDT * 128], BF16)
            wv_sb = const.tile([128, NDT * 128], BF16)
            nc.scalar.dma_start(out=wq_sb[:], in_=wq[:])
            nc.scalar.dma_start(out=wk_sb[:], in_=wk[:])
            nc.scalar.dma_start(out=wv_sb[:], in_=wv[:])
            msk_sb = const.tile([128, 4 * SC], BF16)
            nc.scalar.dma_start(out=msk_sb[:], in_=msk[:])

            qt_tiles = [None] * NCH
            kt_tiles = [None] * NCH
            vc_tiles = [None] * NCH

            # Q/K/V projection for one chunk, as a generator yielding after
            # every PE instruction so it can be drip-fed as filler.
            def proj_gen(c):
                h_c = hsb.tile([128, NDT * SC], BF16, tag="hsb")
                nc.sync.dma_start(out=h_c[:], in_=ht[:, c, :])

                q_c = qt_pool.tile([128, SC], BF16, tag="qt")
                k_c = kt_pool.tile([128, SC], BF16, tag="kt")
                qt_tiles[c] = q_c
                kt_tiles[c] = k_c

                for w_sb, dst in ((wq_sb, q_c), (wk_sb, k_c)):
                    ps = ps_proj.tile([128, SC], F32, tag="psp")
                    for dt in range(NDT):
                        nc.tensor.matmul(
                            ps[:],
                            w_sb[:, dt * 128 : (dt + 1) * 128],
                            h_c[:, dt * SC : (dt + 1) * SC],
                            start=(dt == 0),
                            stop=(dt == NDT - 1),
                        )
                        yield
                    nc.vector.tensor_copy(dst[:], ps[:])

                ps = ps_proj.tile([128, SC], F32, tag="psp")
                for dt in range(NDT):
                    nc.tensor.matmul(
                        ps[:],
                        wv_sb[:, dt * 128 : (dt + 1) * 128],
                        h_c[:, dt * SC : (dt + 1) * SC],
                        start=(dt == 0),
                        stop=(dt == NDT - 1),
                    )
                    yield
                vt_c = vt_pool.tile([128, SC], BF16, tag="vt")
                nc.vector.tensor_copy(vt_c[:], ps[:])

                # PE-transpose V into natural [kpos, dh] layout (+ones col)
                v_c = vc_pool.tile([128, 4 * HPC * (DH + 1)], BF16, tag="vc")
                vc_tiles[c] = v_c
                v_c3 = v_c[:].rearrange("p (a f) -> p a f", a=4 * HPC)
                nc.vector.memset(v_c3[:, :, DH : DH + 1], 1.0)
                for st in range(4):
                    for hh in range(HPC):
                        pst = ps_proj.tile([128, 64], BF16, tag="psp")
                        nc.tensor.transpose(
                            pst[:],
                            vt_c[hh * 64 : (hh + 1) * 64, st * 128 : (st + 1) * 128],
                            ident[hh * 64 : (hh + 1) * 64, :],
                        )
                        base = (st * HPC + hh) * (DH + 1)
                        nc.vector.tensor_copy(v_c[:, base : base + DH], pst[:])
                        yield

            gens = deque(proj_gen(c) for c in range(NCH))

            def run_filler(n):
                done = 0
                while done < n and gens:
                    try:
                        next(gens[0])
                        done += 1
                    except StopIteration:
                        gens.popleft()
                return done

            steps_left = NCH * (3 * NDT + 4 * HPC)  # PE steps in all gens

            # up-front projections for the first chunks
            while len(gens) > NCH - UPFRONT_CHUNKS:
                steps_left -= run_filler(1)

            groups_left = sum(4 * (qc + 1) for qc in range(NCH))

            def emit_av(qc, kt, p_t, o_ps):
                c, t = kt // 4, kt % 4
                nktq = 4 * (qc + 1)
                for hh in range(HPC):
                    base = (t * HPC + hh) * (DH + 1)
                    nc.tensor.matmul(
                        o_ps[hh][:],
                        vc_tiles[c][:, base : base + DH + 1],
                        p_t[:, hh * SC : (hh + 1) * SC],
                        start=(kt == 0),
                        stop=(kt == nktq - 1),
                    )
                if kt == nktq - 1:
                    for hh in range(HPC):
                        stg = stg_pool.tile([DH + 1, SC], F32, tag="stg")
                        nc.vector.tensor_copy(stg[:], o_ps[hh][:])
                        nc.sync.dma_start(
                            out=oden[hh, :, qc * SC : (qc + 1) * SC], in_=stg[:]
                        )

            # AV matmuls lag TWO groups behind the score matmuls so the PE
            # never waits on the exp (ACT) + mask (DVE) chain.
            pending = deque()  # of (qc, kt, p_t, o_ps)
            for qc in range(NCH):
                o_ps = [
                    ps_av.tile([DH + 1, SC], F32, tag="psa", name=f"o_ps_{qc}_{hh}")
                    for hh in range(HPC)
                ]
                nkt = 4 * (qc + 1)
                for kt in range(nkt):
                    c, t = kt // 4, kt % 4
                    s_ps = ps_scores.tile(
                        [128, HPC * SC], F32, tag="pss", name=f"s_ps_{qc}_{kt}"
                    )
                    for hh in range(HPC):
                        nc.tensor.matmul(
                            s_ps[:, hh * SC : (hh + 1) * SC],
                            kt_tiles[c][
                                hh * 64 : (hh + 1) * 64, t * 128 : (t + 1) * 128
                            ],
                            qt_tiles[qc][hh * 64 : (hh + 1) * 64, :],
                            start=True,
                            stop=True,
                        )
                    p_t = p_pool.tile([128, HPC * SC], BF16, tag="pp")
                    nc.scalar.activation(p_t[:], s_ps[:], AF.Exp)
                    if kt >= 4 * qc:  # diagonal tile: zero k > q
                        for hh in range(HPC):
                            nc.vector.tensor_mul(
                                p_t[:, hh * SC : (hh + 1) * SC],
                                p_t[:, hh * SC : (hh + 1) * SC],
                                msk_sb[:, t * SC : (t + 1) * SC],
                            )
                    # drip-feed projection filler to keep the PE gap-free
                    want = 2 if steps_left > groups_left else 1
                    steps_left -= run_filler(want)
                    groups_left -= 1
                    pending.append((qc, kt, p_t, o_ps))
                    if len(pending) > 2:
                        emit_av(*pending.popleft())
            while pending:
                emit_av(*pending.popleft())

    _split_multi_waits(nc)
    return nc


# ---------------------------------------------------------------------------
# Launch 2: sequence-sharded  WO + residual + rmsnorm + MLP + residual
# ---------------------------------------------------------------------------
def build_l2():
    nc = bass.Bass("TRN2", target_bir_lowering=False, debug=False)
    xt = nc.declare_dram_parameter("xt", [128, NDT * SC], BF16, isOutput=False)
    ot = nc.declare_dram_parameter("ot", [128, NDT * SC], BF16, isOutput=False)
    wo = nc.declare_dram_parameter("wo", [128, NDT * NDT * 128], BF16, isOutput=False)
    w1 = nc.declare_dram_parameter("w1", [128, NHT, NDT * 128], BF16, isOutput=False)
    w2 = nc.declare_dram_parameter("w2", [128, NDT, NHT * 128], BF16, isOutput=False)
    b1 = nc.declare_dram_parameter("b1", [128, NHT], F32, isOutput=False)
    b2 = nc.declare_dram_parameter("b2", [128, NDT], F32, isOutput=False)
    yt = nc.declare_dram_parameter("yt", [128, NDT * SC], F32, isOutput=True)

    with tile.TileContext(nc) as tc:
        with (
            tc.tile_pool(name="const", bufs=1) as const,
            tc.tile_pool(name="big", bufs=1) as big,
            tc.tile_pool(name="wt", bufs=6) as wt_pool,
            tc.tile_pool(name="a1", bufs=NHT) as a1_pool,
            tc.tile_pool(name="sq", bufs=2) as sq_pool,
            tc.tile_pool(name="y", bufs=2) as y_pool,
            tc.tile_pool(name="psa", bufs=4, space="PSUM") as ps_a,
            tc.tile_pool(name="psn", bufs=2, space="PSUM") as ps_n,
        ):
            ones_f = const.tile([128, 1], F32)
            nc.vector.memset(ones_f[:], 1.0)
            ones_r = const.tile([128, 1], F32R)
            nc.vector.tensor_copy(ones_r[:], ones_f[:])
            ones_row_f = const.tile([1, 128], F32)
            nc.vector.memset(ones_row_f[:], 1.0)
            ones_row = const.tile([1, 128], F32R)
            nc.vector.tensor_copy(ones_row[:], ones_row_f[:])
            eps_t = const.tile([1, 1], F32)
            nc.vector.memset(eps_t[:], EPS)
            warm_t = const.tile([1, 1], F32)
            # pre-load the Sqrt activation table while ACT is otherwise idle
            nc.scalar.activation(warm_t[:], eps_t[:], AF.Sqrt)
            b1_sb = const.tile([128, NHT], F32)
            b2_sb = const.tile([128, NDT], F32)

            xt_sb = big.tile([128, NDT * SC], BF16)
            ot_sb = big.tile([128, NDT * SC], BF16)
            wo_sb = big.tile([128, NDT * NDT * 128], BF16)
            xm_sb = big.tile([128, NDT * SC], F32)
            h2_sb = big.tile([128, NDT * SC], BF16)
            # ot first on the SP ring (WO matmuls need it), xt in per-tile
            # slices so the residual adds can start before the full load;
            # WO weights stream per-do on the ACT ring in parallel.
            nc.sync.dma_start(out=ot_sb[:], in_=ot[:])
            for dt in range(NDT):
                nc.sync.dma_start(
                    out=xt_sb[:, dt * SC : (dt + 1) * SC],
                    in_=xt[:, dt * SC : (dt + 1) * SC],
                )
            for do in range(NDT):
                nc.scalar.dma_start(
                    out=wo_sb[:, do * NDT * 128 : (do + 1) * NDT * 128],
                    in_=wo[:, do * NDT * 128 : (do + 1) * NDT * 128],
                )
            nc.scalar.dma_start(out=b1_sb[:], in_=b1[:])
            nc.scalar.dma_start(out=b2_sb[:], in_=b2[:])

            # x_mid^T = WO^T @ o^T + x^T, with rmsnorm partial sums interleaved
            ps_sum = ps_n.tile([1, SC], F32, tag="psn")
            for do in range(NDT):
                ps = ps_a.tile([128, SC], F32, tag="psa")
                for od in range(NDT):
                    nc.tensor.matmul(
                        ps[:],
                        wo_sb[:, (do * NDT + od) * 128 : (do * NDT + od + 1) * 128],
                        ot_sb[:, od * SC : (od + 1) * SC],
                        start=(od == 0),
                        stop=(od == NDT - 1),
                    )
                nc.vector.tensor_add(
                    xm_sb[:, do * SC : (do + 1) * SC],
                    ps[:],
                    xt_sb[:, do * SC : (do + 1) * SC],
                )
                sq = sq_pool.tile([128, SC], F32R, tag="sq")
                nc.scalar.activation(
                    sq[:], xm_sb[:, do * SC : (do + 1) * SC], AF.Square
                )
                nc.tensor.matmul(
                    ps_sum[:], ones_r[:], sq[:], start=(do == 0), stop=(do == NDT - 1)
                )

            # rmsnorm2 scale, broadcast to all partitions via ones matmul
            rt = sq_pool.tile([1, SC], F32, tag="rt")
            nc.scalar.activation(
                rt[:], ps_sum[:], AF.Sqrt, bias=eps_t[:], scale=1.0 / D
            )
            rr = sq_pool.tile([1, SC], F32R, tag="rt")
            with nc.allow_low_precision(reason="f32r rounding of rmsnorm scale"):
                nc.vector.reciprocal(rr[:], rt[:])
            ps_b = ps_n.tile([128, SC], F32, tag="psn")
            nc.tensor.matmul(ps_b[:], ones_row[:], rr[:], start=True, stop=True)
            for dt in range(NDT):
                nc.vector.tensor_mul(
                    h2_sb[:, dt * SC : (dt + 1) * SC],
                    xm_sb[:, dt * SC : (dt + 1) * SC],
                    ps_b[:],
                )

            # MLP up + relu (W1 streamed in 8 slabs of 4 hidden tiles,
            # alternating between the SP and ACT HWDGE rings)
            a1_tiles = []
            for g in range(NHT // 4):
                w_t = wt_pool.tile([128, 4 * NDT * 128], BF16, tag="wt")
                eng = nc.sync if g % 2 == 0 else nc.scalar
                eng.dma_start(
                    out=w_t[:].rearrange("p (a f) -> p a f", a=4),
                    in_=w1[:, 4 * g : 4 * (g + 1), :],
                )
                for l in range(4):
                    ht_i = 4 * g + l
                    ps = ps_a.tile([128, SC], F32, tag="psa")
                    for dt in range(NDT):
                        nc.tensor.matmul(
                            ps[:],
                            w_t[
                                :,
                                (l * NDT + dt) * 128 : (l * NDT + dt + 1) * 128,
                            ],
                            h2_sb[:, dt * SC : (dt + 1) * SC],
                            start=(dt == 0),
                            stop=(dt == NDT - 1),
                        )
                    a1 = a1_pool.tile([128, SC], BF16, tag="a1")
                    a1_tiles.append(a1)
                    nc.scalar.activation(
                        a1[:], ps[:], AF.Relu, bias=b1_sb[:, ht_i : ht_i + 1], scale=1.0
                    )

            # MLP down + bias + residual (W2 streamed in 8 slabs)
            for do in range(NDT):
                w_t = wt_pool.tile([128, NHT * 128], BF16, tag="wt")
                eng = nc.sync if do % 2 == 0 else nc.scalar
                eng.dma_start(out=w_t[:], in_=w2[:, do, :])
                ps = ps_a.tile([128, SC], F32, tag="psa")
                for ht_i in range(NHT):
                    nc.tensor.matmul(
                        ps[:],
                        w_t[:, ht_i * 128 : (ht_i + 1) * 128],
                        a1_tiles[ht_i][:],
                        start=(ht_i == 0),
                        stop=(ht_i == NHT - 1),
                    )
                y = y_pool.tile([128, SC], F32, tag="y")
                nc.vector.scalar_tensor_tensor(
                    y[:],
                    ps[:],
                    b2_sb[:, do : do + 1],
                    xm_sb[:, do * SC : (do + 1) * SC],
                    ALU.add,
                    ALU.add,
                )
                nc.scalar.dma_start(out=yt[:, do * SC : (do + 1) * SC], in_=y[:])

    _split_multi_waits(nc)
    return nc


_NC_L1 = None
_NC_L2 = None


def _get_programs():
    global _NC_L1, _NC_L2
    if _NC_L1 is None:
        _NC_L1 = build_l1()
        _NC_L2 = build_l2()
    return _NC_L1, _NC_L2


def _prep_l1(x, g1, WQ, WK, WV):
    ms = (x * x).mean(-1)
    r1 = 1.0 / np.sqrt(ms + EPS)
    h = x * (r1[:, None] * g1[None, :])          # rmsnorm + gain, [S, D]
    hT = np.ascontiguousarray(h.T)               # [D, S]
    # pack [128, chunk, dt*SC]: htp[p, c, dt*SC+j] = hT[dt*128+p, c*SC+j]
    htp = np.ascontiguousarray(
        hT.reshape(NDT, 128, NCH, SC).transpose(1, 2, 0, 3).reshape(128, NCH, NDT * SC)
    ).astype(BF16NP)

    # causal masks for the 4 diagonal k-tile offsets: mask[p, t*SC+j] =
    # (t*128 + p) <= j
    pidx = np.arange(128)[:, None]
    jidx = np.arange(SC)[None, :]
    msk = np.concatenate(
        [((t * 128 + pidx) <= jidx) for t in range(4)], axis=1
    ).astype(BF16NP)

    sq = 1.0 / np.sqrt(DH)
    in_maps = []
    for i in range(NCORES):
        def pack_w(w, scale=1.0):
            # [D, 128] -> [128, dt, 128] partition-major
            return np.ascontiguousarray(
                (w * scale).reshape(NDT, 128, 128).transpose(1, 0, 2).reshape(128, -1)
            ).astype(BF16NP)

        fq = np.concatenate([WQ[2 * i], WQ[2 * i + 1]], axis=1)
        fk = np.concatenate([WK[2 * i], WK[2 * i + 1]], axis=1)
        fv = np.concatenate([WV[2 * i], WV[2 * i + 1]], axis=1)
        in_maps.append(
            {
                "ht": htp,
                "wq": pack_w(fq, sq),
                "wk": pack_w(fk),
                "wv": pack_w(fv),
                "msk": msk,
            }
        )
    return in_maps


def _prep_l2(x, oden_results, g2, WO, W1, B1, W2, B2):
    oT = np.empty((D, S), np.float32)
    for i in range(NCORES):
        od = oden_results[i]["oden"]
        for hh in range(HPC):
            g = 2 * i + hh
            oT[g * DH : (g + 1) * DH] = od[hh, :DH] / od[hh, DH : DH + 1]
    xT = x.T
    w1g = g2[:, None] * W1

    def pack_dmajor(a, rows, cols):
        # [rows*128, cols*128] -> [128, cols, rows, 128]:
        # out[p, c, r, f] = a[r*128+p, c*128+f]
        return np.ascontiguousarray(
            a.reshape(rows, 128, cols, 128).transpose(1, 2, 0, 3)
        ).astype(BF16NP)

    wo_p = pack_dmajor(WO, NDT, NDT).reshape(128, -1)
    w1_p = pack_dmajor(w1g, NDT, NHT).reshape(128, NHT, NDT * 128)
    w2_p = pack_dmajor(W2, NHT, NDT).reshape(128, NDT, NHT * 128)
    b1r = np.ascontiguousarray(B1.reshape(NHT, 128).T)
    b2r = np.ascontiguousarray(B2.reshape(NDT, 128).T)

    in_maps = []
    for i in range(NCORES):
        sl = slice(i * SC, (i + 1) * SC)
        xts = xT[:, sl]   # [D, SC]
        ots = oT[:, sl]
        xtp = np.ascontiguousarray(
            xts.reshape(NDT, 128, SC).transpose(1, 0, 2).reshape(128, -1)
        ).astype(BF16NP)
        otp = np.ascontiguousarray(
            ots.reshape(NDT, 128, SC).transpose(1, 0, 2).reshape(128, -1)
        ).astype(BF16NP)
        in_maps.append(
            {
                "xt": xtp,
                "ot": otp,
                "wo": wo_p,
                "w1": w1_p,
                "w2": w2_p,
                "b1": b1r,
                "b2": b2r,
            }
        )
    return in_maps


def kernel(**inputs):
    x = np.asarray(inputs["x"], dtype=np.float32)[0]
    g1 = np.asarray(inputs["g1"], dtype=np.float32)
    g2 = np.asarray(inputs["g2"], dtype=np.float32)
    WQ = np.asarray(inputs["WQ"], dtype=np.float32)
    WK = np.asarray(inputs["WK"], dtype=np.float32)
    WV = np.asarray(inputs["WV"], dtype=np.float32)
    WO = np.asarray(inputs["WO"], dtype=np.float32)
    W1 = np.asarray(inputs["W1"], dtype=np.float32)
    B1 = np.asarray(inputs["B1"], dtype=np.float32)
    W2 = np.asarray(inputs["W2"], dtype=np.float32)
    B2 = np.asarray(inputs["B2"], dtype=np.float32)

    nc1, nc2 = _get_programs()
    core_ids = list(range(NCORES))

    in1 = _prep_l1(x, g1, WQ, WK, WV)
    res1 = run_bass_kernel_spmd(nc1, in1, core_ids).results

    in2 = _prep_l2(x, res1, g2, WO, W1, B1, W2, B2)
    res2 = run_bass_kernel_spmd(nc2, in2, core_ids).results

    yT = np.empty((D, S), np.float32)
    for i in range(NCORES):
        yt = res2[i]["yt"]  # [128, NDT*SC]
        yT[:, i * SC : (i + 1) * SC] = (
            yt.reshape(128, NDT, SC).transpose(1, 0, 2).reshape(D, SC)
        )
    return np.ascontiguousarray(yT.T).reshape(1, S, D)


# revision 24
# speedup vs baseline: 1.6942x; 1.0340x over previous
"""Trainium2 Bass kernel for nn_DecoderBlock (B=1, S=4096, D=1024, H=16, dh=64).

Strategy (8 NeuronCores, SPMD, no collectives):
  Launch 1 — attention, tensor-parallel over heads (2 heads/core). Host
    pre-computes hT = rmsnorm(x)*g1 transposed (the norm scale depends only
    on x) and folds 1/sqrt(dh) into the per-core WQ head slices. Each core
    computes Q^T/K^T/V^T projections, a causal streaming softmax (no max
    subtraction — scores are O(1) for this problem), and the per-head
    unnormalized attention output o_u^T plus softmax denominators, all in
    transposed [feature, seq] space (V is transposed on the PE via identity
    matmuls).

    The instruction stream is software-pipelined so the PE never idles
    (keeps the HAM clock gate at 2.4 GHz): attention-value matmuls lag one
    k-tile group behind the score matmuls (covering the exp latency on the
    ACT engine), and the Q/K/V projections of later chunks are drip-fed
    into the attention stream as PE filler work. Causal masking of the
    diagonal tiles is a DVE multiply with 4 static mask tiles.
  Host — concatenates per-head o_u^T, divides by denominators.
  Launch 2 — o@WO + residual, rmsnorm2, MLP, residual; sequence-parallel
    (512 tokens/core). Weights are streamed as bf16, host-prepacked into
    partition-contiguous SBUF layout (2KB+ DMA lines). RMSNorm partial sums
    are interleaved with the WO matmuls to shrink the serial bubble.

Matmuls run in bf16/f32r (1 cycle/row at moving-dim 512); accumulation is
f32 in PSUM. Residual paths (x, x_mid, y) stay f32.
"""

import sys
import types
from collections import deque

import numpy as np
import ml_dtypes

BF16NP = ml_dtypes.bfloat16


# ---------------------------------------------------------------------------
# Environment compatibility shims (inlined — kernel.py must be self-contained)
# ---------------------------------------------------------------------------
def _install_compat():
    try:
        import trn_agent_boot.trn_boot as _tb

        if "antenv.axon_hooks" not in sys.modules:
            _hook = _tb._ntff_profile_via_ctypes("/opt/axon/libaxon_pjrt.so")
            _m = types.ModuleType("antenv.axon_hooks")
            _m.get_axon_ntff_profile_hook = lambda: _hook
            sys.modules["antenv.axon_hooks"] = _m
    except Exception:
        pass

    import concourse.mybir as mybir
    from concourse import tile as _tile
    from concourse import bass_utils as _bass_utils
    from concourse.vector_clock import ScopedClock as _ScopedClock

    _bass_utils.upload_artifacts = lambda tmpdir: f"local:{tmpdir}"

    def _patched_drain_and_barrier(self, tick_clock, wait_clock):
        nc = self.nc
        drain_inst = nc.sync.drain()
        wait_clock.add_sem_waits(
            drain_inst.ins, _ScopedClock({None: tick_clock.global_clock})
        )
        si = drain_inst.ins.sync_info
        waits = list(si.on_wait or []) if si else []
        if len(waits) > 1:
            drain_inst.ins.sync_info = mybir.SyncInfo(
                on_wait=waits[:1], on_update=list(si.on_update or [])
            )
            for i in range(1, len(waits)):
                nop = nc.sync.nop(nofuse=True, hint="drain_wait_split")
                nop.ins.sync_info = mybir.SyncInfo(on_wait=waits[i : i + 1], on_update=[])
        nc.all_engine_barrier()
        assert self.sems is not None
        popped = nc._tile_sem_poison_stack.pop()
        assert popped is self._sem_poison
        nc.clear_and_free_semaphores(list(self.sems.allocated().values()))
        nc.all_engine_barrier()

    _tile.TileContext._drain_and_barrier = _patched_drain_and_barrier


_install_compat()

import concourse.bass as bass
import concourse.mybir as mybir
from concourse import tile
from concourse.masks import make_identity
from concourse.bass_utils import run_bass_kernel_spmd

F32 = mybir.dt.float32
F32R = mybir.dt.float32r
BF16 = mybir.dt.bfloat16
AF = mybir.ActivationFunctionType
ALU = mybir.AluOpType

S, D, H, DH = 4096, 1024, 16, 64
NCORES = 8
HPC = H // NCORES          # heads per core = 2
SC = S // NCORES           # seq chunk per core = 512
NCH = S // SC              # number of 512-chunks = 8
NDT = D // 128             # d-tiles = 8
NHT = 4 * D // 128         # hidden tiles = 32
EPS = 1e-6

# chunks projected up-front before the attention stream starts; the rest are
# drip-fed as PE filler between attention groups
UPFRONT_CHUNKS = 4


def _split_multi_waits(nc, max_waits: int = 1):
    """This walrus build accepts only one sem-wait per instruction; hoist
    extras onto fresh NoOps inserted just before, on the same engine."""
    n_split = 0
    for fn in nc.m.functions:
        for blk in fn.blocks:
            out = []
            changed = False
            for inst in blk.instructions:
                si = inst.sync_info
                waits = list(si.on_wait or []) if si else []
                if len(waits) > max_waits:
                    changed = True
                    for i in range(0, len(waits) - max_waits, max_waits):
                        nop = mybir.InstNoOp(
                            name=f"I-waitsplit-{n_split}", ins=[], outs=[]
                        )
                        n_split += 1
                        nop.engine = inst.engine
                        nop.sync_info = mybir.SyncInfo(
                            on_wait=waits[i : i + max_waits], on_update=[]
                        )
                        out.append(nop)
                    inst.sync_info = mybir.SyncInfo(
                        on_wait=waits[len(waits) - max_waits :],
                        on_update=list(si.on_update or []),
                    )
                out.append(inst)
            if changed:
                blk.instructions = out
    return n_split


# ---------------------------------------------------------------------------
# Launch 1: head-sharded attention
# ---------------------------------------------------------------------------
def build_l1():
    nc = bass.Bass("TRN2", target_bir_lowering=False, debug=False)
    ht = nc.declare_dram_parameter("ht", [128, NCH, NDT * SC], BF16, isOutput=False)
    wq = nc.declare_dram_parameter("wq", [128, NDT * 128], BF16, isOutput=False)
    wk = nc.declare_dram_parameter("wk", [128, NDT * 128], BF16, isOutput=False)
    wv = nc.declare_dram_parameter("wv", [128, NDT * 128], BF16, isOutput=False)
    msk = nc.declare_dram_parameter("msk", [128, 4 * SC], BF16, isOutput=False)
    oden = nc.declare_dram_parameter("oden", [HPC, DH + 1, S], F32, isOutput=True)

    with tile.TileContext(nc) as tc:
        with (
            tc.tile_pool(name="const", bufs=1) as const,
            tc.tile_pool(name="hsb", bufs=3) as hsb,
            tc.tile_pool(name="qt", bufs=NCH) as qt_pool,
            tc.tile_pool(name="kt", bufs=NCH) as kt_pool,
            tc.tile_pool(name="vc", bufs=NCH) as vc_pool,
            tc.tile_pool(name="vt", bufs=2) as vt_pool,
            tc.tile_pool(name="pp", bufs=3) as p_pool,
            tc.tile_pool(name="stg", bufs=2) as stg_pool,
            tc.tile_pool(name="psp", bufs=2, space="PSUM") as ps_proj,
            tc.tile_pool(name="pss", bufs=2, space="PSUM") as ps_scores,
            tc.tile_pool(name="psa", bufs=2, space="PSUM") as ps_av,
        ):
            # constants: identity replicated in both partition halves so
            # head-1 transposes (base_partition 64) can use a matching slice
            idf = const.tile([128, 64], F32)
            make_identity(nc, idf[0:64, :])
            make_identity(nc, idf[64:128, :])
            ident = const.tile([128, 64], BF16)
            nc.vector.tensor_copy(ident[:], idf[:])

            # weights + masks on the ACT HWDGE ring so they land in parallel
            # with the first hT chunk DMAs on the SP ring
            wq_sb = const.tile([128, NDT * 128], BF16)
            wk_sb = const.tile([128, NDT * 128], BF16)
            wv_sb = const.tile([128, NDT * 128], BF16)
            nc.scalar.dma_start(out=wq_sb[:], in_=wq[:])
            nc.scalar.dma_start(out=wk_sb[:], in_=wk[:])
            nc.scalar.dma_start(out=wv_sb[:], in_=wv[:])
            msk_sb = const.tile([128, 4 * SC], BF16)
            nc.scalar.dma_start(out=msk_sb[:], in_=msk[:])

            qt_tiles = [None] * NCH
            kt_tiles = [None] * NCH
            vc_tiles = [None] * NCH

            # Q/K/V projection for one chunk, as a generator yielding after
            # every PE instruction so it can be drip-fed as filler.
            def proj_gen(c):
                h_c = hsb.tile([128, NDT * SC], BF16, tag="hsb")
                nc.sync.dma_start(out=h_c[:], in_=ht[:, c, :])

                q_c = qt_pool.tile([128, SC], BF16, tag="qt")
                k_c = kt_pool.tile([128, SC], BF16, tag="kt")
                qt_tiles[c] = q_c
                kt_tiles[c] = k_c

                for w_sb, dst in ((wq_sb, q_c), (wk_sb, k_c)):
                    ps = ps_proj.tile([128, SC], F32, tag="psp")
                    for dt in range(NDT):
                        nc.tensor.matmul(
                            ps[:],
                            w_sb[:, dt * 128 : (dt + 1) * 128],
                            h_c[:, dt * SC : (dt + 1) * SC],
                            start=(dt == 0),
                            stop=(dt == NDT - 1),
                        )
                        yield
                    nc.vector.tensor_copy(dst[:], ps[:])

                ps = ps_proj.tile([128, SC], F32, tag="psp")
                for dt in range(NDT):
                    nc.tensor.matmul(
                        ps[:],
                        wv_sb[:, dt * 128 : (dt + 1) * 128],
                        h_c[:, dt * SC : (dt + 1) * SC],
                        start=(dt == 0),
                        stop=(dt == NDT - 1),
                    )
                    yield
                vt_c = vt_pool.tile([128, SC], BF16, tag="vt")
                nc.vector.tensor_copy(vt_c[:], ps[:])

                # PE-transpose V into natural [kpos, dh] layout (+ones col)
                v_c = vc_pool.tile([128, 4 * HPC * (DH + 1)], BF16, tag="vc")
                vc_tiles[c] = v_c
                v_c3 = v_c[:].rearrange("p (a f) -> p a f", a=4 * HPC)
                nc.vector.memset(v_c3[:, :, DH : DH + 1], 1.0)
                for st in range(4):
                    for hh in range(HPC):
                        pst = ps_proj.tile([128, 64], BF16, tag="psp")
                        nc.tensor.transpose(
                            pst[:],
                            vt_c[hh * 64 : (hh + 1) * 64, st * 128 : (st + 1) * 128],
                            ident[hh * 64 : (hh + 1) * 64, :],
                        )
                        base = (st * HPC + hh) * (DH + 1)
                        nc.vector.tensor_copy(v_c[:, base : base + DH], pst[:])
                        yield

            gens = deque(proj_gen(c) for c in range(NCH))

            def run_filler(n):
                done = 0
                while done < n and gens:
                    try:
                        next(gens[0])
                        done += 1
                    except StopIteration:
                        gens.popleft()
                return done

            steps_left = NCH * (3 * NDT + 4 * HPC)  # PE steps in all gens

            # up-front projections for the first chunks
            while len(gens) > NCH - UPFRONT_CHUNKS:
                steps_left -= run_filler(1)

            groups_left = sum(4 * (qc + 1) for qc in range(NCH))

            def emit_av(qc, kt, p_t, o_ps):
                c, t = kt // 4, kt % 4
                nktq = 4 * (qc + 1)
                j0 = t * 128 if kt >= 4 * qc else 0
                for hh in range(HPC):
                    base = (t * HPC + hh) * (DH + 1)
                    nc.tensor.matmul(
                        o_ps[hh][:, j0:SC],
                        vc_tiles[c][:, base : base + DH + 1],
                        p_t[:, hh * SC + j0 : (hh + 1) * SC],
                        start=(kt == 0),
                        stop=(kt == nktq - 1),
                    )
                if kt == nktq - 1:
                    for hh in range(HPC):
                        stg = stg_pool.tile([DH + 1, SC], F32, tag="stg")
                        nc.vector.tensor_copy(stg[:], o_ps[hh][:])
                        nc.sync.dma_start(
                            out=oden[hh, :, qc * SC : (qc + 1) * SC], in_=stg[:]
                        )

            # AV matmuls lag TWO groups behind the score matmuls so the PE
            # never waits on the exp (ACT) + mask (DVE) chain.
            pending = deque()  # of (qc, kt, p_t, o_ps)
            for qc in range(NCH):
                o_ps = [
                    ps_av.tile([DH + 1, SC], F32, tag="psa", name=f"o_ps_{qc}_{hh}")
                    for hh in range(HPC)
                ]
                nkt = 4 * (qc + 1)
                for kt in range(nkt):
                    c, t = kt // 4, kt % 4
                    # diagonal tiles: columns j < t*128 are fully masked —
                    # skip them in scores, exp, mask, and AV
                    j0 = t * 128 if kt >= 4 * qc else 0
                    s_ps = ps_scores.tile(
                        [128, HPC * SC], F32, tag="pss", name=f"s_ps_{qc}_{kt}"
                    )
                    for hh in range(HPC):
                        nc.tensor.matmul(
                            s_ps[:, hh * SC + j0 : (hh + 1) * SC],
                            kt_tiles[c][
                                hh * 64 : (hh + 1) * 64, t * 128 : (t + 1) * 128
                            ],
                            qt_tiles[qc][hh * 64 : (hh + 1) * 64, j0:SC],
                            start=True,
                            stop=True,
                        )
                    p_t = p_pool.tile([128, HPC * SC], BF16, tag="pp")
                    if j0:
                        sv = s_ps[:].rearrange("p (h j) -> p h j", h=HPC)[:, :, j0:]
                        pv = p_t[:].rearrange("p (h j) -> p h j", h=HPC)[:, :, j0:]
                        nc.scalar.activation(pv, sv, AF.Exp)
                    else:
                        nc.scalar.activation(p_t[:], s_ps[:], AF.Exp)
                    if kt >= 4 * qc:  # diagonal tile: zero k > q
                        for hh in range(HPC):
                            nc.vector.tensor_mul(
                                p_t[:, hh * SC + j0 : (hh + 1) * SC],
                                p_t[:, hh * SC + j0 : (hh + 1) * SC],
                                msk_sb[:, t * SC + j0 : (t + 1) * SC],
                            )
                    # drip-feed projection filler to keep the PE gap-free
                    want = 2 if steps_left > groups_left else 1
                    steps_left -= run_filler(want)
                    groups_left -= 1
                    pending.append((qc, kt, p_t, o_ps))
                    if len(pending) > 2:
                        emit_av(*pending.popleft())
            while pending:
                emit_av(*pending.popleft())

    _split_multi_waits(nc)
    return nc


# ---------------------------------------------------------------------------
# Launch 2: sequence-sharded  WO + residual + rmsnorm + MLP + residual
# ---------------------------------------------------------------------------
def build_l2():
    nc = bass.Bass("TRN2", target_bir_lowering=False, debug=False)
    xt = nc.declare_dram_parameter("xt", [128, NDT * SC], BF16, isOutput=False)
    ot = nc.declare_dram_parameter("ot", [128, NDT * SC], BF16, isOutput=False)
    wo = nc.declare_dram_parameter("wo", [128, NDT * NDT * 128], BF16, isOutput=False)
    w1 = nc.declare_dram_parameter("w1", [128, NHT, NDT * 128], BF16, isOutput=False)
    w2 = nc.declare_dram_parameter("w2", [128, NDT, NHT * 128], BF16, isOutput=False)
    b1 = nc.declare_dram_parameter("b1", [128, NHT], F32, isOutput=False)
    b2 = nc.declare_dram_parameter("b2", [128, NDT], F32, isOutput=False)
    yt = nc.declare_dram_parameter("yt", [128, NDT * SC], F32, isOutput=True)

    with tile.TileContext(nc) as tc:
        with (
            tc.tile_pool(name="const", bufs=1) as const,
            tc.tile_pool(name="big", bufs=1) as big,
            tc.tile_pool(name="wt", bufs=6) as wt_pool,
            tc.tile_pool(name="a1", bufs=NHT) as a1_pool,
            tc.tile_pool(name="sq", bufs=2) as sq_pool,
            tc.tile_pool(name="y", bufs=2) as y_pool,
            tc.tile_pool(name="psa", bufs=4, space="PSUM") as ps_a,
            tc.tile_pool(name="psn", bufs=2, space="PSUM") as ps_n,
        ):
            ones_f = const.tile([128, 1], F32)
            nc.vector.memset(ones_f[:], 1.0)
            ones_r = const.tile([128, 1], F32R)
            nc.vector.tensor_copy(ones_r[:], ones_f[:])
            ones_row_f = const.tile([1, 128], F32)
            nc.vector.memset(ones_row_f[:], 1.0)
            ones_row = const.tile([1, 128], F32R)
            nc.vector.tensor_copy(ones_row[:], ones_row_f[:])
            eps_t = const.tile([1, 1], F32)
            nc.vector.memset(eps_t[:], EPS)
            warm_t = const.tile([1, 1], F32)
            # pre-load the Sqrt activation table while ACT is otherwise idle
            nc.scalar.activation(warm_t[:], eps_t[:], AF.Sqrt)
            b1_sb = const.tile([128, NHT], F32)
            b2_sb = const.tile([128, NDT], F32)

            xt_sb = big.tile([128, NDT * SC], BF16)
            ot_sb = big.tile([128, NDT * SC], BF16)
            wo_sb = big.tile([128, NDT * NDT * 128], BF16)
            xm_sb = big.tile([128, NDT * SC], F32)
            xmb_sb = big.tile([128, NDT * SC], BF16)
            # ot first on the SP ring (WO matmuls need it), xt in per-tile
            # slices so the residual adds can start before the full load;
            # WO weights stream per-do on the ACT ring in parallel.
            nc.sync.dma_start(out=ot_sb[:], in_=ot[:])
            for dt in range(NDT):
                nc.sync.dma_start(
                    out=xt_sb[:, dt * SC : (dt + 1) * SC],
                    in_=xt[:, dt * SC : (dt + 1) * SC],
                )
            for do in range(NDT):
                nc.scalar.dma_start(
                    out=wo_sb[:, do * NDT * 128 : (do + 1) * NDT * 128],
                    in_=wo[:, do * NDT * 128 : (do + 1) * NDT * 128],
                )
            nc.scalar.dma_start(out=b1_sb[:], in_=b1[:])
            nc.scalar.dma_start(out=b2_sb[:], in_=b2[:])

            # x_mid^T = WO^T @ o^T + x^T, with rmsnorm partial sums interleaved
            ps_sum = ps_n.tile([1, SC], F32, tag="psn")
            for do in range(NDT):
                ps = ps_a.tile([128, SC], F32, tag="psa")
                for od in range(NDT):
                    nc.tensor.matmul(
                        ps[:],
                        wo_sb[:, (do * NDT + od) * 128 : (do * NDT + od + 1) * 128],
                        ot_sb[:, od * SC : (od + 1) * SC],
                        start=(od == 0),
                        stop=(od == NDT - 1),
                    )
                nc.vector.tensor_add(
                    xm_sb[:, do * SC : (do + 1) * SC],
                    ps[:],
                    xt_sb[:, do * SC : (do + 1) * SC],
                )
                # bf16 copy for the W1 moving operand (gpsimd is idle here)
                nc.gpsimd.tensor_copy(
                    xmb_sb[:, do * SC : (do + 1) * SC],
                    xm_sb[:, do * SC : (do + 1) * SC],
                )
                sq = sq_pool.tile([128, SC], F32R, tag="sq")
                nc.scalar.activation(
                    sq[:], xm_sb[:, do * SC : (do + 1) * SC], AF.Square
                )
                nc.tensor.matmul(
                    ps_sum[:], ones_r[:], sq[:], start=(do == 0), stop=(do == NDT - 1)
                )

            # rmsnorm2 scale: computed while W1 matmuls already run on the
            # UNNORMALIZED xm (the per-token scale commutes with the matmul);
            # the scale is applied on DVE to the W1 outputs before the relu.
            rt = sq_pool.tile([1, SC], F32, tag="rt")
            nc.scalar.activation(
                rt[:], ps_sum[:], AF.Sqrt, bias=eps_t[:], scale=1.0 / D
            )
            rr = sq_pool.tile([1, SC], F32R, tag="rt")
            with nc.allow_low_precision(reason="f32r rounding of rmsnorm scale"):
                nc.vector.reciprocal(rr[:], rt[:])

            # MLP up + scale + relu (W1 streamed in 8 slabs of 4 hidden
            # tiles, alternating between the SP and ACT HWDGE rings)
            a1_tiles = []
            ps_b = None
            for g in range(NHT // 4):
                w_t = wt_pool.tile([128, 4 * NDT * 128], BF16, tag="wt")
                eng = nc.sync if g % 2 == 0 else nc.scalar
                eng.dma_start(
                    out=w_t[:].rearrange("p (a f) -> p a f", a=4),
                    in_=w1[:, 4 * g : 4 * (g + 1), :],
                )
                for l in range(4):
                    ht_i = 4 * g + l
                    ps = ps_a.tile([128, SC], F32, tag="psa")
                    for dt in range(NDT):
                        nc.tensor.matmul(
                            ps[:],
                            w_t[
                                :,
                                (l * NDT + dt) * 128 : (l * NDT + dt + 1) * 128,
                            ],
                            xmb_sb[:, dt * SC : (dt + 1) * SC],
                            start=(dt == 0),
                            stop=(dt == NDT - 1),
                        )
                    if ps_b is None:
                        # broadcast 1/rms to all partitions via ones matmul;
                        # emitted after the first W1 group so the PE never
                        # waits on the sqrt/reciprocal chain
                        ps_b = ps_n.tile([128, SC], F32, tag="psn")
                        nc.tensor.matmul(
                            ps_b[:], ones_row[:], rr[:], start=True, stop=True
                        )
                        rb_sb = big.tile([128, SC], F32)
                        nc.vector.tensor_copy(rb_sb[:], ps_b[:])
                    zsc = sq_pool.tile([128, SC], BF16, tag="zs")
                    nc.vector.tensor_mul(zsc[:], ps[:], rb_sb[:])
                    a1 = a1_pool.tile([128, SC], BF16, tag="a1")
                    a1_tiles.append(a1)
                    nc.scalar.activation(
                        a1[:], zsc[:], AF.Relu, bias=b1_sb[:, ht_i : ht_i + 1],
                        scale=1.0,
                    )

            # MLP down + bias + residual (W2 streamed in 8 slabs)
            for do in range(NDT):
                w_t = wt_pool.tile([128, NHT * 128], BF16, tag="wt")
                eng = nc.sync if do % 2 == 0 else nc.scalar
                eng.dma_start(out=w_t[:], in_=w2[:, do, :])
                ps = ps_a.tile([128, SC], F32, tag="psa")
                for ht_i in range(NHT):
                    nc.tensor.matmul(
                        ps[:],
                        w_t[:, ht_i * 128 : (ht_i + 1) * 128],
                        a1_tiles[ht_i][:],
                        start=(ht_i == 0),
                        stop=(ht_i == NHT - 1),
                    )
                y = y_pool.tile([128, SC], F32, tag="y")
                nc.vector.scalar_tensor_tensor(
                    y[:],
                    ps[:],
                    b2_sb[:, do : do + 1],
                    xm_sb[:, do * SC : (do + 1) * SC],
                    ALU.add,
                    ALU.add,
                )
                nc.scalar.dma_start(out=yt[:, do * SC : (do + 1) * SC], in_=y[:])

    _split_multi_waits(nc)
    return nc


_NC_L1 = None
_NC_L2 = None


def _get_programs():
    global _NC_L1, _NC_L2
    if _NC_L1 is None:
        _NC_L1 = build_l1()
        _NC_L2 = build_l2()
    return _NC_L1, _NC_L2


def _prep_l1(x, g1, WQ, WK, WV):
    ms = (x * x).mean(-1)
    r1 = 1.0 / np.sqrt(ms + EPS)
    h = x * (r1[:, None] * g1[None, :])          # rmsnorm + gain, [S, D]
    hT = np.ascontiguousarray(h.T)               # [D, S]
    # pack [128, chunk, dt*SC]: htp[p, c, dt*SC+j] = hT[dt*128+p, c*SC+j]
    htp = np.ascontiguousarray(
        hT.reshape(NDT, 128, NCH, SC).transpose(1, 2, 0, 3).reshape(128, NCH, NDT * SC)
    ).astype(BF16NP)

    # causal masks for the 4 diagonal k-tile offsets: mask[p, t*SC+j] =
    # (t*128 + p) <= j
    pidx = np.arange(128)[:, None]
    jidx = np.arange(SC)[None, :]
    msk = np.concatenate(
        [((t * 128 + pidx) <= jidx) for t in range(4)], axis=1
    ).astype(BF16NP)

    sq = 1.0 / np.sqrt(DH)
    in_maps = []
    for i in range(NCORES):
        def pack_w(w, scale=1.0):
            # [D, 128] -> [128, dt, 128] partition-major
            return np.ascontiguousarray(
                (w * scale).reshape(NDT, 128, 128).transpose(1, 0, 2).reshape(128, -1)
            ).astype(BF16NP)

        fq = np.concatenate([WQ[2 * i], WQ[2 * i + 1]], axis=1)
        fk = np.concatenate([WK[2 * i], WK[2 * i + 1]], axis=1)
        fv = np.concatenate([WV[2 * i], WV[2 * i + 1]], axis=1)
        in_maps.append(
            {
                "ht": htp,
                "wq": pack_w(fq, sq),
                "wk": pack_w(fk),
                "wv": pack_w(fv),
                "msk": msk,
            }
        )
    return in_maps


def _prep_l2(x, oden_results, g2, WO, W1, B1, W2, B2):
    oT = np.empty((D, S), np.float32)
    for i in range(NCORES):
        od = oden_results[i]["oden"]
        for hh in range(HPC):
            g = 2 * i + hh
            oT[g * DH : (g + 1) * DH] = od[hh, :DH] / od[hh, DH : DH + 1]
    xT = x.T
    w1g = g2[:, None] * W1

    def pack_dmajor(a, rows, cols):
        # [rows*128, cols*128] -> [128, cols, rows, 128]:
        # out[p, c, r, f] = a[r*128+p, c*128+f]
        return np.ascontiguousarray(
            a.reshape(rows, 128, cols, 128).transpose(1, 2, 0, 3)
        ).astype(BF16NP)

    wo_p = pack_dmajor(WO, NDT, NDT).reshape(128, -1)
    w1_p = pack_dmajor(w1g, NDT, NHT).reshape(128, NHT, NDT * 128)
    w2_p = pack_dmajor(W2, NHT, NDT).reshape(128, NDT, NHT * 128)
    b1r = np.ascontiguousarray(B1.reshape(NHT, 128).T)
    b2r = np.ascontiguousarray(B2.reshape(NDT, 128).T)

    in_maps = []
    for i in range(NCORES):
        sl = slice(i * SC, (i + 1) * SC)
        xts = xT[:, sl]   # [D, SC]
        ots = oT[:, sl]
        xtp = np.ascontiguousarray(
            xts.reshape(NDT, 128, SC).transpose(1, 0, 2).reshape(128, -1)
        ).astype(BF16NP)
        otp = np.ascontiguousarray(
            ots.reshape(NDT, 128, SC).transpose(1, 0, 2).reshape(128, -1)
        ).astype(BF16NP)
        in_maps.append(
            {
                "xt": xtp,
                "ot": otp,
                "wo": wo_p,
                "w1": w1_p,
                "w2": w2_p,
                "b1": b1r,
                "b2": b2r,
            }
        )
    return in_maps


def kernel(**inputs):
    x = np.asarray(inputs["x"], dtype=np.float32)[0]
    g1 = np.asarray(inputs["g1"], dtype=np.float32)
    g2 = np.asarray(inputs["g2"], dtype=np.float32)
    WQ = np.asarray(inputs["WQ"], dtype=np.float32)
    WK = np.asarray(inputs["WK"], dtype=np.float32)
    WV = np.asarray(inputs["WV"], dtype=np.float32)
    WO = np.asarray(inputs["WO"], dtype=np.float32)
    W1 = np.asarray(inputs["W1"], dtype=np.float32)
    B1 = np.asarray(inputs["B1"], dtype=np.float32)
    W2 = np.asarray(inputs["W2"], dtype=np.float32)
    B2 = np.asarray(inputs["B2"], dtype=np.float32)

    nc1, nc2 = _get_programs()
    core_ids = list(range(NCORES))

    in1 = _prep_l1(x, g1, WQ, WK, WV)
    res1 = run_bass_kernel_spmd(nc1, in1, core_ids).results

    in2 = _prep_l2(x, res1, g2, WO, W1, B1, W2, B2)
    res2 = run_bass_kernel_spmd(nc2, in2, core_ids).results

    yT = np.empty((D, S), np.float32)
    for i in range(NCORES):
        yt = res2[i]["yt"]  # [128, NDT*SC]
        yT[:, i * SC : (i + 1) * SC] = (
            yt.reshape(128, NDT, SC).transpose(1, 0, 2).reshape(D, SC)
        )
    return np.ascontiguousarray(yT.T).reshape(1, S, D)
